# revision 20
# baseline (speedup 1.0000x reference)
"""Trainium2 Bass kernel for one GPT-style transformer block.

Problem: B=8, T=1024, C=768, NH=12 heads (HD=64), pre-LN attention + MLP,
key-padding mask, tanh-gelu.  Sharding: data-parallel over batch — each of
the 8 NeuronCores processes one batch element end-to-end (no collectives).

Per-core dataflow:
  - Attention matmuls run in fp8-e4m3 with DoubleRow perf mode (two 128-row
    k-tiles contracted per instruction, 2x PE throughput vs bf16); the MLP
    stays bf16 (fp8 there would blow the 2e-2 error budget; attention's
    contribution to the output is small so its fp8 noise is damped).
  - x resident token-major fp32 [128p, 8t, 768] (residual stream)
  - LN1 on token-major tiles -> bf16 -> PE-transpose -> h1T fp8 feature-major
  - q^T,k^T computed feature-major via DR matmuls (lhsT=W_attn chunk pairs,
    rhs=h1T chunk pairs).  W_attn's q/k columns are PERMUTED host-side so
    that head h's dims 0-31 and 32-63 land in adjacent feature chunks at
    partitions 32*(h%4): a [32p, 2, N] slice of qT/kT is then a legal
    DoubleRow operand pair contracting the full head dim (64).
  - v computed token-major (DR over feature-chunk pairs), stored per-head as
    v_ext fp8 [128p, head, kt, 65] with a ones-column (col 64) so the
    attention row-sum (softmax denominator) falls out of the same matmul.
    The key-padding mask is folded in by zeroing masked v_ext rows entirely.
  - scores TRANSPOSED per head: s^T[k, q] via DR (lhsT = kT [32,2,128],
    rhs = qT [32,2,512]); exp(s/8)*2^-5 on the scalar engine (bias -5ln2
    keeps the heavy-tailed exp inside fp8 range, max ~178 < 240; the 2^-5
    hits numerator and denominator alike so softmax cancels it), stored fp8.
    Heads are software-pipelined: exp(h+3) overlaps av(h) (sT triple-buffered).
  - Key compaction: kernel() permutes each batch's tokens so unmasked keys
    come first; with <=640 unmasked keys the last 3 of 8 key chunks are
    entirely masked and skipped.  Runtime guard falls back otherwise.
  - AV: o[tq, 65] = sum_kt s^T[kt,tq].T @ v_ext[kt], DR over kt-chunk pairs
    (2 pairs + 1 plain fp8 tail); per-token softmax normalization is a
    per-partition scalar multiply.
  - o -> PE-transpose -> o^T fp8; proj via DR; residual add into x (fp32).
  - LN2 -> h2T bf16; a^T = gelu(W_fc^T @ h2T) feature-major; fc2 token-major;
    residual add; DMA out.  (MLP all bf16.)

Two program variants: `trivial=True` (unit LN gains, zero biases — the
distribution setup_inputs() generates) skips all bias/gain work; the general
variant applies them.  kernel() picks per call based on the actual inputs.
"""

import math

import numpy as np
import ml_dtypes

import concourse.bass as bass
import concourse.mybir as mybir
import concourse.tile as tile
from concourse import bacc
from concourse.bass import ds, ts
from concourse.masks import make_identity

F32 = mybir.dt.float32
BF16 = mybir.dt.bfloat16
FP8 = mybir.dt.float8e4
AF = mybir.ActivationFunctionType
ALU = mybir.AluOpType
DR = mybir.MatmulPerfMode.DoubleRow

T, C, NH, HD = 1024, 768, 12, 64
TT = T // 128          # 8 token tiles
CC = C // 128          # 6 feature chunks
FC = (4 * C) // 128    # 24 ffn-hidden chunks
N_CORES = 8
EPS = 1e-5
EXP_BIAS = -5.0 * math.log(2.0)   # exp output scaled 2^-5: fits fp8e4 range


def _bcast(ap_1d: bass.AP, p: int = 128) -> bass.AP:
    """Broadcast a 1-D DRAM AP across p partitions (zero partition stride)."""
    return bass.AP(tensor=ap_1d.tensor, offset=ap_1d.offset, ap=[[0, p]] + ap_1d.ap)


def build_bass(
    repeat: int = 1,
    trivial: bool = True,
    kt_chunks: int = 8,
    dr_qkv: bool = True,
    dr_scores: bool = True,
    dr_av: bool = True,
    dr_proj: bool = True,
    mlp_split: bool = False,
) -> bass.Bass:
    """kt_chunks: number of 128-key chunks attention processes (keys beyond
    kt_chunks*128 must be masked — kernel() permutes unmasked keys first and
    guards the count).  8 = full attention.

    dr_*: use fp8 DoubleRow for that stage's matmuls; stages without DR run
    in bf16 exactly like the original baseline (plain fp8 is SLOWER than
    bf16 on this hardware — measured ~1.5x — so never plain-fp8).
    dr_scores=False emits baseline-style [64,128] per-head slices, which
    require UNPERMUTED w_attn (kernel() permutes iff DR_SCORES)."""
    KT = kt_chunks
    qk_dt = FP8 if dr_scores else BF16    # scores operands
    sv_dt = FP8 if dr_av else BF16        # AV operands (sT, v_ext)
    exp_bias = EXP_BIAS if dr_av else None  # fp8 sT needs the 2^-5 range shift
    # Bacc (not plain Bass): its compile() runs generate_event_semaphores,
    # which splits multi-wait instructions — HW allows 1 wait per instruction.
    nc = bacc.Bacc(None)

    x_d = nc.dram_tensor("x", [T, C], F32, kind="ExternalInput")
    mask_d = nc.dram_tensor("mask01", [T], F32, kind="ExternalInput")
    # w_attn arrives PERMUTED (q/k columns regrouped for DoubleRow scores)
    wattn_d = nc.dram_tensor("w_attn", [C, 3 * C], FP8, kind="ExternalInput")
    wproj_d = nc.dram_tensor("w_proj", [C, C], FP8, kind="ExternalInput")
    if mlp_split:
        # hi = fp8(16*W), lo = fp8((16*W - hi)*16); out rescaled by 1/16 twice
        wfchi_d = nc.dram_tensor("w_fc_hi", [C, 4 * C], FP8, kind="ExternalInput")
        wfclo_d = nc.dram_tensor("w_fc_lo", [C, 4 * C], FP8, kind="ExternalInput")
        wfc2hi_d = nc.dram_tensor("w_fc2_hi", [4 * C, C], FP8, kind="ExternalInput")
        wfc2lo_d = nc.dram_tensor("w_fc2_lo", [4 * C, C], FP8, kind="ExternalInput")
    else:
        wfc_d = nc.dram_tensor("w_fc", [C, 4 * C], BF16, kind="ExternalInput")
        wfc2_d = nc.dram_tensor("w_fc2", [4 * C, C], BF16, kind="ExternalInput")
    if not trivial:
        ln1g_d = nc.dram_tensor("ln1_g", [C], F32, kind="ExternalInput")
        ln1b_d = nc.dram_tensor("ln1_b", [C], F32, kind="ExternalInput")
        ln2g_d = nc.dram_tensor("ln2_g", [C], F32, kind="ExternalInput")
        ln2b_d = nc.dram_tensor("ln2_b", [C], F32, kind="ExternalInput")
        battn_d = nc.dram_tensor("b_attn", [3 * C], F32, kind="ExternalInput")
        bproj_d = nc.dram_tensor("b_proj", [C], F32, kind="ExternalInput")
        bfc_d = nc.dram_tensor("b_fc", [4 * C], F32, kind="ExternalInput")
        bfc2_d = nc.dram_tensor("b_fc2", [C], F32, kind="ExternalInput")
    out_d = nc.dram_tensor("out", [T, C], F32, kind="ExternalOutput")

    with tile.TileContext(nc) as tc:
        from contextlib import ExitStack

        with ExitStack() as ctx:
            consts = ctx.enter_context(tc.tile_pool(name="consts", bufs=1))
            xpool = ctx.enter_context(tc.tile_pool(name="xpool", bufs=1))
            htmp_pool = ctx.enter_context(tc.tile_pool(name="htmp", bufs=3))
            stat_pool = ctx.enter_context(tc.tile_pool(name="stats", bufs=6))
            hT2_pool = ctx.enter_context(tc.tile_pool(name="hT2", bufs=1))
            wproj_pool = ctx.enter_context(tc.tile_pool(name="wproj", bufs=1))
            ps_mm = ctx.enter_context(tc.tile_pool(name="ps_mm", bufs=2, space="PSUM"))
            ps_tr = ctx.enter_context(tc.tile_pool(name="ps_tr", bufs=2, space="PSUM"))

            # ---------------- constants ----------------
            ident = consts.tile([128, 128], BF16, name="ident")
            make_identity(nc, ident)
            mask_col = consts.tile([128, TT], F32, name="mask_col")
            nc.gpsimd.dma_start(out=mask_col, in_=mask_d[:].rearrange("(t p) -> p t", p=128))
            eps_t = consts.tile([128, 1], F32, name="eps_t")
            nc.vector.memset(eps_t, EPS)
            expb_t = consts.tile([128, 1], F32, name="expb_t")
            nc.vector.memset(expb_t, EXP_BIAS)
            sixt_t = consts.tile([128, 1], F32, name="sixt_t")
            nc.vector.memset(sixt_t, 1.0 / 16.0)

            if not trivial:
                g1_bc = consts.tile([128, C], F32, name="g1_bc")
                b1_bc = consts.tile([128, C], F32, name="b1_bc")
                g2_bc = consts.tile([128, C], F32, name="g2_bc")
                b2_bc = consts.tile([128, C], F32, name="b2_bc")
                battnv_bc = consts.tile([128, C], F32, name="battnv_bc")
                bproj_bc = consts.tile([128, C], F32, name="bproj_bc")
                bfc2_bc = consts.tile([128, C], F32, name="bfc2_bc")
                nc.gpsimd.dma_start(out=g1_bc, in_=_bcast(ln1g_d[:]))
                nc.gpsimd.dma_start(out=b1_bc, in_=_bcast(ln1b_d[:]))
                nc.gpsimd.dma_start(out=g2_bc, in_=_bcast(ln2g_d[:]))
                nc.gpsimd.dma_start(out=b2_bc, in_=_bcast(ln2b_d[:]))
                nc.gpsimd.dma_start(out=battnv_bc, in_=_bcast(battn_d[ds(2 * C, C)]))
                nc.gpsimd.dma_start(out=bproj_bc, in_=_bcast(bproj_d[:]))
                nc.gpsimd.dma_start(out=bfc2_bc, in_=_bcast(bfc2_d[:]))
                # b_attn q/k biases arrive PERMUTED like the w_attn columns
                battn_qk = consts.tile([128, 12], F32, name="battn_qk")
                nc.gpsimd.dma_start(
                    out=battn_qk,
                    in_=battn_d[ds(0, 2 * C)].rearrange("(m p) -> p m", p=128),
                )
                bfc_col = consts.tile([128, FC], F32, name="bfc_col")
                nc.gpsimd.dma_start(
                    out=bfc_col, in_=bfc_d[:].rearrange("(m p) -> p m", p=128)
                )

            def layer_norm_to_hT(x_slice, g_bc, b_bc, hT, t, hT_lo=None):
                """LN over C (free dim) of one token tile; write transpose
                into hT[:, c, t*128:...] via PE transposes (copies on ScalarE).
                hT dtype (fp8 for h1T, bf16 for h2T) set by the copy cast."""
                stats = stat_pool.tile([128, 2, 6], F32, name="stats", tag="lnstats")
                for i in range(2):
                    nc.vector.bn_stats(out=stats[:, i, :], in_=x_slice[:, ts(i, 384)])
                mv = stat_pool.tile([128, 2], F32, name="mv", tag="lnmv")
                nc.vector.bn_aggr(out=mv, in_=stats)
                rstd = stat_pool.tile([128, 1], F32, name="rstd", tag="rstd")
                nc.scalar.activation(out=rstd, in_=mv[:, 1:2], func=AF.Sqrt, bias=eps_t[:, 0:1])
                nc.vector.reciprocal(rstd, rstd)
                hbf = htmp_pool.tile([128, C], BF16, name="hbf", tag="hbf")
                if trivial:
                    nc.vector.tensor_scalar(
                        out=hbf, in0=x_slice, scalar1=mv[:, 0:1], scalar2=rstd,
                        op0=ALU.subtract, op1=ALU.mult,
                    )
                else:
                    htmp = htmp_pool.tile([128, C], F32, name="htmp", tag="htmp")
                    nc.vector.tensor_scalar(
                        out=htmp, in0=x_slice, scalar1=mv[:, 0:1], scalar2=rstd,
                        op0=ALU.subtract, op1=ALU.mult,
                    )
                    nc.vector.tensor_mul(htmp, htmp, g_bc)
                    nc.vector.tensor_add(hbf, htmp, b_bc)
                for c in range(CC):
                    ptr = ps_tr.tile([128, 128], BF16, name="ptr", tag="tr")
                    nc.tensor.transpose(ptr, hbf[:, ts(c, 128)], ident)
                    nc.scalar.copy(hT[:, c, ts(t, 128)], ptr)
                    if hT_lo is not None:
                        # subnormal-fp8 residual of the fp8 cast (no rescale)
                        nc.vector.tensor_sub(
                            hT_lo[:, c, ts(t, 128)], ptr, hT[:, c, ts(t, 128)]
                        )

            # ================= one full block (repeatable) =================
            for _rep in range(repeat):
              # residual stream, token-major fp32 (per-tile DMAs so LN1 can
              # start as soon as the first rows land)
              x_t = xpool.tile([128, TT, C], F32, name="x_t", tag="x_t")
              for t in range(TT):
                  nc.sync.dma_start(out=x_t[:, t, :], in_=x_d[ts(t, 128), :])

              # ---- attention scope ----
              with ExitStack() as actx:
                wattn_pool = actx.enter_context(tc.tile_pool(name="wattn", bufs=1))
                ps_s = actx.enter_context(tc.tile_pool(name="ps_s", bufs=2, space="PSUM"))
                hT_pool = actx.enter_context(tc.tile_pool(name="hT1", bufs=1))
                qkT_pool = actx.enter_context(tc.tile_pool(name="qkT", bufs=1))
                vext_pool = actx.enter_context(tc.tile_pool(name="vext", bufs=1))
                sT_pool = actx.enter_context(tc.tile_pool(name="sT", bufs=3))
                opool = actx.enter_context(tc.tile_pool(name="opool", bufs=1))

                wattn_sb = wattn_pool.tile([128, CC, 3 * C], FP8, name="wattn_sb")
                nc.gpsimd.dma_start(
                    out=wattn_sb, in_=wattn_d[:, :].rearrange("(c p) n -> p c n", p=128)
                )
                wproj_sb = wproj_pool.tile([128, CC, C], FP8, name="wproj_sb")
                nc.gpsimd.dma_start(
                    out=wproj_sb, in_=wproj_d[:, :].rearrange("(c p) n -> p c n", p=128)
                )

                h1T = hT_pool.tile([128, CC, T], FP8, name="h1T", tag="hT")
                qT = qkT_pool.tile([128, CC, T], qk_dt, name="qT", tag="qT")
                kT = qkT_pool.tile([128, CC, KT * 128], qk_dt, name="kT", tag="kT")
                vext = vext_pool.tile([128, NH, KT, HD + 1], sv_dt, name="vext")
                nc.vector.memset(vext[:, :, :, HD : HD + 1], 1.0)
                if trivial:
                    # mask the ones column up front (v columns get the mask
                    # folded into their psum->sbuf copies below)
                    for t in range(KT):
                        nc.vector.tensor_scalar_mul(
                            out=vext[:, :, t, HD : HD + 1],
                            in0=vext[:, :, t, HD : HD + 1],
                            scalar1=mask_col[:, t : t + 1],
                        )

                # q^T / k^T feature-major, emitted token-half by token-half
                # so the first half's matmuls run while LN1 of tiles 4-7 is
                # still computing (the PE stream is in-order)
                def qk_phase(nqi):
                    for m in range(12):
                        dest = qT if m < 6 else kT
                        nlim = T if m < 6 else KT * 128
                        n0 = nqi * 512
                        nsz = min(512, nlim - n0)
                        if nsz <= 0:
                            continue
                        pq = ps_mm.tile([128, nsz], F32, name="pq", tag="mm")
                        if dr_qkv:
                            for c in range(0, CC, 2):
                                nc.tensor.matmul(
                                    pq,
                                    lhsT=wattn_sb[:, c : c + 2, ts(m, 128)],
                                    rhs=h1T[:, c : c + 2, ds(n0, nsz)],
                                    start=(c == 0),
                                    stop=(c == CC - 2),
                                    perf_mode=DR,
                                )
                        else:
                            for c in range(CC):
                                nc.tensor.matmul(
                                    pq,
                                    lhsT=wattn_sb[:, c, ts(m, 128)],
                                    rhs=h1T[:, c, ds(n0, nsz)],
                                    start=(c == 0),
                                    stop=(c == CC - 1),
                                )
                        if trivial:
                            nc.vector.tensor_copy(dest[:, m % 6, ds(n0, nsz)], pq)
                        else:
                            nc.scalar.activation(
                                out=dest[:, m % 6, ds(n0, nsz)], in_=pq,
                                func=AF.Identity, bias=battn_qk[:, m : m + 1],
                            )

                for t in range(4):
                    layer_norm_to_hT(
                        x_t[:, t, :],
                        None if trivial else g1_bc,
                        None if trivial else b1_bc,
                        h1T, t,
                    )
                qk_phase(0)
                for t in range(4, TT):
                    layer_norm_to_hT(
                        x_t[:, t, :],
                        None if trivial else g1_bc,
                        None if trivial else b1_bc,
                        h1T, t,
                    )
                qk_phase(1)

                sT_tiles = {}

                def emit_scores(h):
                    # scores^T for head h: DoubleRow over the head dim (64 =
                    # 32 partitions x 2 chunk-slots, from the host-side W_attn
                    # column permutation); one big [128, 2048] exp per pair of
                    # kt chunks (amortizes the ACT op overhead)
                    sT = sT_pool.tile([128, KT, T], sv_dt, name="sT", tag="sT")
                    sT_tiles[h] = sT
                    for kt in range(KT):
                        pss = ps_s.tile([128, 2, 512], F32, name="pss", tag="ss")
                        for nq in range(2):
                            if dr_scores:
                                g, i = divmod(h, 4)
                                r0 = 32 * i
                                nc.tensor.matmul(
                                    pss[:, nq, :],
                                    lhsT=kT[ds(r0, 32), ds(2 * g, 2), ts(kt, 128)],
                                    rhs=qT[ds(r0, 32), ds(2 * g, 2), ts(nq, 512)],
                                    start=True,
                                    stop=True,
                                    perf_mode=DR,
                                    # base_partition() rejects 96; rows<=32 may
                                    # sit at any quadrant via explicit position
                                    tile_position=(r0, 0),
                                )
                            else:
                                hc, hr = divmod(h, 2)
                                r0 = hr * 64
                                nc.tensor.matmul(
                                    pss[:, nq, :],
                                    lhsT=kT[ds(r0, 64), hc, ts(kt, 128)],
                                    rhs=qT[ds(r0, 64), hc, ts(nq, 512)],
                                    start=True,
                                    stop=True,
                                )
                        nc.scalar.activation(
                            out=sT[:, kt, :],
                            in_=pss.rearrange("p a b -> p (a b)"),
                            func=AF.Exp,
                            scale=0.125,
                            bias=0.0 if exp_bias is None else expb_t[:, 0:1],
                        )

                # prime the head pipeline (sT is triple-buffered)
                emit_scores(0)
                emit_scores(1)
                emit_scores(2)

                # v token-major, scattered per head into v_ext (batched copies)
                for t in range(KT):
                    for n0, nsz in ((0, 512), (512, 256)):
                        pv = ps_mm.tile([128, nsz], F32, name="pv", tag="mm")
                        if dr_qkv:
                            for c in range(0, CC, 2):
                                nc.tensor.matmul(
                                    pv,
                                    lhsT=h1T[:, c : c + 2, ts(t, 128)],
                                    rhs=wattn_sb[:, c : c + 2, ds(2 * C + n0, nsz)],
                                    start=(c == 0),
                                    stop=(c == CC - 2),
                                    perf_mode=DR,
                                )
                        else:
                            for c in range(CC):
                                nc.tensor.matmul(
                                    pv,
                                    lhsT=h1T[:, c, ts(t, 128)],
                                    rhs=wattn_sb[:, c, ds(2 * C + n0, nsz)],
                                    start=(c == 0),
                                    stop=(c == CC - 1),
                                )
                        h0, h1 = n0 // HD, (n0 + nsz) // HD
                        pv_h = pv.rearrange("p (h d) -> p h d", d=HD)
                        if trivial:
                            # mask folded into the psum->sbuf copy
                            nc.vector.tensor_scalar_mul(
                                out=vext[:, h0:h1, t, 0:HD], in0=pv_h,
                                scalar1=mask_col[:, t : t + 1],
                            )
                        else:
                            nc.vector.tensor_add(
                                out=vext[:, h0:h1, t, 0:HD], in0=pv_h,
                                in1=battnv_bc[:, ds(n0, nsz)].rearrange(
                                    "p (h d) -> p h d", d=HD
                                ),
                            )
                if not trivial:
                    # fold the key-padding mask into v_ext (incl. ones col)
                    for t in range(KT):
                        nc.vector.tensor_scalar_mul(
                            out=vext[:, :, t, :], in0=vext[:, :, t, :],
                            scalar1=mask_col[:, t : t + 1],
                        )

                o_t = opool.tile([128, TT, C], BF16, name="o_t", tag="op")
                oT = qkT_pool.tile([128, CC, T], FP8, name="oT", tag="oT")
                KT_PAIRS = KT // 2
                for h in range(NH):
                    sT = sT_tiles.pop(h)
                    for tq in range(TT):
                        pav = ps_mm.tile([128, HD + 1], F32, name="pav", tag="mm")
                        if dr_av:
                            for j in range(KT_PAIRS):
                                nc.tensor.matmul(
                                    pav,
                                    lhsT=sT[:, 2 * j : 2 * j + 2, ts(tq, 128)],
                                    rhs=vext[:, h, 2 * j : 2 * j + 2, :],
                                    start=(j == 0),
                                    stop=(KT % 2 == 0 and j == KT_PAIRS - 1),
                                    perf_mode=DR,
                                )
                            if KT % 2 == 1:
                                nc.tensor.matmul(
                                    pav,
                                    lhsT=sT[:, KT - 1, ts(tq, 128)],
                                    rhs=vext[:, h, KT - 1, :],
                                    start=False,
                                    stop=True,
                                )
                        else:
                            for kt in range(KT):
                                nc.tensor.matmul(
                                    pav,
                                    lhsT=sT[:, kt, ts(tq, 128)],
                                    rhs=vext[:, h, kt, :],
                                    start=(kt == 0),
                                    stop=(kt == KT - 1),
                                )
                        rec = stat_pool.tile([128, 1], F32, name="rec", tag="rec")
                        nc.vector.reciprocal(rec, pav[:, HD : HD + 1])
                        nc.vector.tensor_scalar_mul(
                            out=o_t[:, tq, ts(h, HD)], in0=pav[:, 0:HD], scalar1=rec
                        )
                    if h + 3 < NH:
                        emit_scores(h + 3)
                    if h % 2 == 1:
                        # heads 2c,2c+1 done -> transpose o chunk c now (PE has
                        # slack while ACT exps; copies on DVE, not the busy ACT)
                        c = h // 2
                        for t in range(TT):
                            ptr = ps_tr.tile([128, 128], BF16, name="ptr2", tag="tr")
                            nc.tensor.transpose(ptr, o_t[:, t, ts(c, 128)], ident)
                            nc.vector.tensor_copy(oT[:, c, ts(t, 128)], ptr)

                # proj + residual into x (fp32); LN2 + h2T per 4-tile group
                # so fcT's first half (needs h2T tiles 0-3) starts while the
                # second half of proj still runs
                h2_dt = FP8 if mlp_split else BF16
                h2T = hT2_pool.tile([128, CC, T], h2_dt, name="h2T", tag="hT2")
                h2T_lo = (
                    hT2_pool.tile([128, CC, T], FP8, name="h2Tlo", tag="hT2lo")
                    if mlp_split else None
                )
                for grp in range(2):
                    for t in range(grp * 4, grp * 4 + 4):
                        for n0, nsz in ((0, 512), (512, 256)):
                            pp = ps_mm.tile([128, nsz], F32, name="pp", tag="mm")
                            if dr_proj:
                                for c in range(0, CC, 2):
                                    nc.tensor.matmul(
                                        pp,
                                        lhsT=oT[:, c : c + 2, ts(t, 128)],
                                        rhs=wproj_sb[:, c : c + 2, ds(n0, nsz)],
                                        start=(c == 0),
                                        stop=(c == CC - 2),
                                        perf_mode=DR,
                                    )
                            else:
                                for c in range(CC):
                                    nc.tensor.matmul(
                                        pp,
                                        lhsT=oT[:, c, ts(t, 128)],
                                        rhs=wproj_sb[:, c, ds(n0, nsz)],
                                        start=(c == 0),
                                        stop=(c == CC - 1),
                                    )
                            if not trivial:
                                nc.vector.tensor_add(pp, pp, bproj_bc[:, ds(n0, nsz)])
                            nc.vector.tensor_add(
                                x_t[:, t, ds(n0, nsz)], x_t[:, t, ds(n0, nsz)], pp
                            )
                    for t in range(grp * 4, grp * 4 + 4):
                        layer_norm_to_hT(
                            x_t[:, t, :],
                            None if trivial else g2_bc,
                            None if trivial else b2_bc,
                            h2T, t, hT_lo=h2T_lo,
                        )

              # ---- MLP scope ----
              with ExitStack() as mctx:
                aT_pool = mctx.enter_context(tc.tile_pool(name="aT", bufs=1))
                outsb_pool = mctx.enter_context(tc.tile_pool(name="outsb", bufs=2))

                if not mlp_split:
                    wfc_pool = mctx.enter_context(tc.tile_pool(name="wfc", bufs=1))
                    wfc2_pool = mctx.enter_context(tc.tile_pool(name="wfc2", bufs=1))
                    wfc_sb = wfc_pool.tile([128, CC, 4 * C], BF16, name="wfc_sb")
                    nc.gpsimd.dma_start(
                        out=wfc_sb, in_=wfc_d[:, :].rearrange("(c p) n -> p c n", p=128)
                    )
                    wfc2_sb = wfc2_pool.tile([128, FC, C], BF16, name="wfc2_sb")
                    nc.gpsimd.dma_start(
                        out=wfc2_sb, in_=wfc2_d[:, :].rearrange("(m p) n -> p m n", p=128)
                    )

                    for half in range(2):
                        aT = aT_pool.tile([128, FC, 512], BF16, name="aT", tag="aT")
                        for m in range(FC):
                            pf = ps_mm.tile([128, 512], F32, name="pf", tag="mm")
                            for c in range(CC):
                                nc.tensor.matmul(
                                    pf,
                                    lhsT=wfc_sb[:, c, ts(m, 128)],
                                    rhs=h2T[:, c, ds(half * 512, 512)],
                                    start=(c == 0),
                                    stop=(c == CC - 1),
                                )
                            nc.scalar.activation(
                                out=aT[:, m, :], in_=pf, func=AF.Gelu_apprx_tanh,
                                bias=0.0 if trivial else bfc_col[:, m : m + 1],
                            )
                        for i in range(4):
                            t = half * 4 + i
                            outsb = outsb_pool.tile([128, C], F32, name="outsb", tag="outsb")
                            for n0, nsz in ((0, 512), (512, 256)):
                                pf2 = ps_mm.tile([128, nsz], F32, name="pf2", tag="mm")
                                for m in range(FC):
                                    nc.tensor.matmul(
                                        pf2,
                                        lhsT=aT[:, m, ts(i, 128)],
                                        rhs=wfc2_sb[:, m, ds(n0, nsz)],
                                        start=(m == 0),
                                        stop=(m == FC - 1),
                                    )
                                if not trivial:
                                    nc.vector.tensor_add(pf2, pf2, bfc2_bc[:, ds(n0, nsz)])
                                nc.vector.tensor_add(
                                    outsb[:, ds(n0, nsz)], x_t[:, t, ds(n0, nsz)], pf2
                                )
                            nc.sync.dma_start(out=out_d[ts(t, 128), :], in_=outsb)
                else:
                    # split-fp8 MLP: W stored as hi=fp8(16W), lo=fp8((16W-hi)*16).
                    # P1 accumulates scale-16 terms (hi@hi and act_lo@hi: act lo
                    # is the UNSCALED subnormal-fp8 cast residual), P2 the
                    # scale-256 term (hi@lo); combined = P1 + P2/16, and the
                    # final 1/16 folds into the gelu scale / output rescale.
                    wfc_pool = mctx.enter_context(tc.tile_pool(name="wfc", bufs=1))
                    wfc2_pool = mctx.enter_context(tc.tile_pool(name="wfc2", bufs=1))
                    a8_pool = mctx.enter_context(tc.tile_pool(name="a8", bufs=1))
                    ps_mlp = mctx.enter_context(
                        tc.tile_pool(name="ps_mlp", bufs=4, space="PSUM")
                    )
                    wfchi_sb = wfc_pool.tile([128, CC, 4 * C], FP8, name="wfchi_sb")
                    wfclo_sb = wfc_pool.tile([128, CC, 4 * C], FP8, name="wfclo_sb")
                    nc.gpsimd.dma_start(
                        out=wfchi_sb, in_=wfchi_d[:, :].rearrange("(c p) n -> p c n", p=128)
                    )
                    nc.gpsimd.dma_start(
                        out=wfclo_sb, in_=wfclo_d[:, :].rearrange("(c p) n -> p c n", p=128)
                    )
                    wfc2hi_sb = wfc2_pool.tile([128, FC, C], FP8, name="wfc2hi_sb")
                    wfc2lo_sb = wfc2_pool.tile([128, FC, C], FP8, name="wfc2lo_sb")
                    nc.gpsimd.dma_start(
                        out=wfc2hi_sb, in_=wfc2hi_d[:, :].rearrange("(m p) n -> p m n", p=128)
                    )
                    nc.gpsimd.dma_start(
                        out=wfc2lo_sb, in_=wfc2lo_d[:, :].rearrange("(m p) n -> p m n", p=128)
                    )

                    for half in range(2):
                        aT = aT_pool.tile([128, FC, 512], BF16, name="aT", tag="aT")
                        ahi = a8_pool.tile([128, FC, 512], FP8, name="ahi", tag="ahi")
                        alo = a8_pool.tile([128, FC, 512], FP8, name="alo", tag="alo")
                        hcols = ds(half * 512, 512)
                        for m in range(FC):
                            p1 = ps_mlp.tile([128, 512], F32, name="p1", tag="mmp")
                            for li, (wsb, hT_) in enumerate(
                                ((wfchi_sb, h2T), (wfclo_sb, h2T), (wfchi_sb, h2T_lo))
                            ):
                                for c in range(0, CC, 2):
                                    nc.tensor.matmul(
                                        p1,
                                        lhsT=wsb[:, c : c + 2, ts(m, 128)],
                                        rhs=hT_[:, c : c + 2, hcols],
                                        start=(li == 0 and c == 0),
                                        stop=(li == 2 and c == CC - 2),
                                        perf_mode=DR,
                                    )
                            nc.scalar.activation(
                                out=aT[:, m, :], in_=p1, func=AF.Gelu_apprx_tanh,
                                scale=1.0 / 16.0,
                                bias=0.0 if trivial else bfc_col[:, m : m + 1],
                            )
                            nc.vector.tensor_copy(ahi[:, m, :], aT[:, m, :])
                            nc.vector.tensor_sub(alo[:, m, :], aT[:, m, :], ahi[:, m, :])
                        for i in range(4):
                            t = half * 4 + i
                            outsb = outsb_pool.tile([128, C], F32, name="outsb", tag="outsb")
                            for n0, nsz in ((0, 512), (512, 256)):
                                q1 = ps_mlp.tile([128, nsz], F32, name="q1", tag="mmp")
                                for li, (aT_, wsb) in enumerate(
                                    ((ahi, wfc2hi_sb), (ahi, wfc2lo_sb), (alo, wfc2hi_sb))
                                ):
                                    for m in range(0, FC, 2):
                                        nc.tensor.matmul(
                                            q1,
                                            lhsT=aT_[:, m : m + 2, ts(i, 128)],
                                            rhs=wsb[:, m : m + 2, ds(n0, nsz)],
                                            start=(li == 0 and m == 0),
                                            stop=(li == 2 and m == FC - 2),
                                            perf_mode=DR,
                                        )
                                nc.vector.tensor_scalar_mul(
                                    outsb[:, ds(n0, nsz)], q1, sixt_t[:, 0:1]
                                )
                                nc.vector.tensor_add(
                                    outsb[:, ds(n0, nsz)],
                                    outsb[:, ds(n0, nsz)],
                                    x_t[:, t, ds(n0, nsz)],
                                )
                                if not trivial:
                                    nc.vector.tensor_add(
                                        outsb[:, ds(n0, nsz)],
                                        outsb[:, ds(n0, nsz)],
                                        bfc2_bc[:, ds(n0, nsz)],
                                    )
                            nc.sync.dma_start(out=out_d[ts(t, 128), :], in_=outsb)

    return nc


_NC_CACHE = {}

COMPACT_KT = 5  # attention processes 5*128 = 640 keys; guarded in kernel()

# Graded configuration: DoubleRow only where hardware-verified faster.
DR_QKV = True
DR_SCORES = False
DR_AV = False
DR_PROJ = True
MLP_SPLIT = True


def _get_nc(trivial: bool = True, kt_chunks: int = COMPACT_KT) -> bass.Bass:
    key = (trivial, kt_chunks)
    if key not in _NC_CACHE:
        nc = build_bass(
            trivial=trivial, kt_chunks=kt_chunks,
            dr_qkv=DR_QKV, dr_scores=DR_SCORES, dr_av=DR_AV, dr_proj=DR_PROJ,
            mlp_split=MLP_SPLIT,
        )
        nc.finalize()
        _NC_CACHE[key] = nc
    return _NC_CACHE[key]


TRACE = False
LAST_RESULTS = None
LAST_IN_MAPS = None


def _permute_qk_cols(w_qk: np.ndarray) -> np.ndarray:
    """Reorder q-or-k columns [.., 768] so head h's dims 0-31 / 32-63 land in
    feature chunks 2*(h//4) / 2*(h//4)+1 at partitions 32*(h%4)."""
    perm = np.empty(C, np.int64)
    pos = 0
    for g in range(3):           # head groups of 4
        for half in range(2):    # dims 0-31 | 32-63
            for i in range(4):   # head within group
                h = 4 * g + i
                perm[pos : pos + 32] = h * HD + half * 32 + np.arange(32)
                pos += 32
    return w_qk[..., perm]


def kernel(**inputs) -> np.ndarray:
    global LAST_RESULTS, LAST_IN_MAPS

    f32 = lambda a: np.ascontiguousarray(np.asarray(a, dtype=np.float32))
    bf = lambda a: np.ascontiguousarray(
        np.asarray(a, dtype=np.float32).astype(ml_dtypes.bfloat16)
    )
    f8 = lambda a: np.ascontiguousarray(
        np.asarray(a, dtype=np.float32).astype(ml_dtypes.float8_e4m3)
    )

    x = f32(inputs["x"])                       # [8, 1024, 768]
    mask = np.asarray(inputs["attn_mask"])     # [8, 1024] int32

    lng1, lnb1 = f32(inputs["ln1_g"]), f32(inputs["ln1_b"])
    lng2, lnb2 = f32(inputs["ln2_g"]), f32(inputs["ln2_b"])
    ba, bp = f32(inputs["b_attn"]), f32(inputs["b_proj"])
    bf_, bf2 = f32(inputs["b_fc"]), f32(inputs["b_fc2"])
    trivial = bool(
        (lng1 == 1).all() and (lnb1 == 0).all() and (lng2 == 1).all()
        and (lnb2 == 0).all() and (ba == 0).all() and (bp == 0).all()
        and (bf_ == 0).all() and (bf2 == 0).all()
    )

    # Key compaction: permute tokens per batch so unmasked keys come first.
    # Attention is permutation-equivariant over keys, and LN/MLP/residual are
    # per-token, so permuting rows of x and un-permuting the output is exact.
    # With <= COMPACT_KT*128 unmasked keys the remaining key chunks are all
    # masked (zero contribution) and can be skipped entirely.
    mask01 = (mask != 0)
    counts = mask01.sum(axis=1)
    compact = bool(counts.max() <= COMPACT_KT * 128)
    kt_chunks = COMPACT_KT if compact else TT

    perms = []
    for b in range(N_CORES):
        perm = np.argsort(~mask01[b], kind="stable")  # unmasked first
        perms.append(perm)

    nc = _get_nc(trivial, kt_chunks)

    # q/k column permutation for DoubleRow scores (see _permute_qk_cols)
    W_attn = f32(inputs["W_attn"])
    if DR_SCORES:
        W_attn_perm = np.concatenate(
            [
                _permute_qk_cols(W_attn[:, 0:C]),
                _permute_qk_cols(W_attn[:, C : 2 * C]),
                W_attn[:, 2 * C :],
            ],
            axis=1,
        )
    else:
        W_attn_perm = W_attn
    common = {
        "w_attn": f8(W_attn_perm),
        "w_proj": f8(inputs["W_proj"]),
    }
    if MLP_SPLIT:
        def _wsplit(w):
            ws = f32(w) * 16.0
            hi = np.asarray(ws, np.float32).astype(ml_dtypes.float8_e4m3)
            lo = ws - hi.astype(np.float32)   # raw residual (subnormal fp8)
            return np.ascontiguousarray(hi), np.ascontiguousarray(
                lo.astype(ml_dtypes.float8_e4m3)
            )
        common["w_fc_hi"], common["w_fc_lo"] = _wsplit(inputs["W_fc"])
        common["w_fc2_hi"], common["w_fc2_lo"] = _wsplit(inputs["W_fc2"])
    else:
        common["w_fc"] = bf(inputs["W_fc"])
        common["w_fc2"] = bf(inputs["W_fc2"])
    if not trivial:
        if DR_SCORES:
            ba_perm = np.concatenate(
                [_permute_qk_cols(ba[0:C]), _permute_qk_cols(ba[C : 2 * C]), ba[2 * C :]]
            )
        else:
            ba_perm = ba
        common.update(
            ln1_g=lng1, ln1_b=lnb1, ln2_g=lng2, ln2_b=lnb2,
            b_attn=ba_perm, b_proj=bp, b_fc=bf_, b_fc2=bf2,
        )
    in_maps = []
    for b in range(N_CORES):
        m = dict(common)
        m["x"] = np.ascontiguousarray(x[b][perms[b]])
        m["mask01"] = np.ascontiguousarray(mask01[b][perms[b]].astype(np.float32))
        in_maps.append(m)

    from concourse.bass_utils import run_bass_kernel_spmd

    LAST_IN_MAPS = in_maps
    res = run_bass_kernel_spmd(nc, in_maps, core_ids=list(range(N_CORES)), trace=TRACE)
    LAST_RESULTS = res
    out = np.empty((N_CORES, T, C), np.float32)
    for b in range(N_CORES):
        out[b, perms[b]] = np.asarray(res.results[b]["out"])
    return out


# revision 21
# speedup vs baseline: 1.3128x; 1.3128x over previous
"""Trainium2 Bass kernel for one GPT-style transformer block.

Problem: B=8, T=1024, C=768, NH=12 heads (HD=64), pre-LN attention + MLP,
key-padding mask, tanh-gelu.  Sharding: data-parallel over batch — each of
the 8 NeuronCores processes one batch element end-to-end (no collectives).

Per-core dataflow:
  - Attention matmuls run in fp8-e4m3 with DoubleRow perf mode (two 128-row
    k-tiles contracted per instruction, 2x PE throughput vs bf16); the MLP
    stays bf16 (fp8 there would blow the 2e-2 error budget; attention's
    contribution to the output is small so its fp8 noise is damped).
  - x resident token-major fp32 [128p, 8t, 768] (residual stream)
  - LN1 on token-major tiles -> bf16 -> PE-transpose -> h1T fp8 feature-major
  - q^T,k^T computed feature-major via DR matmuls (lhsT=W_attn chunk pairs,
    rhs=h1T chunk pairs).  W_attn's q/k columns are PERMUTED host-side so
    that head h's dims 0-31 and 32-63 land in adjacent feature chunks at
    partitions 32*(h%4): a [32p, 2, N] slice of qT/kT is then a legal
    DoubleRow operand pair contracting the full head dim (64).
  - v computed token-major (DR over feature-chunk pairs), stored per-head as
    v_ext fp8 [128p, head, kt, 65] with a ones-column (col 64) so the
    attention row-sum (softmax denominator) falls out of the same matmul.
    The key-padding mask is folded in by zeroing masked v_ext rows entirely.
  - scores TRANSPOSED per head: s^T[k, q] via DR (lhsT = kT [32,2,128],
    rhs = qT [32,2,512]); exp(s/8)*2^-5 on the scalar engine (bias -5ln2
    keeps the heavy-tailed exp inside fp8 range, max ~178 < 240; the 2^-5
    hits numerator and denominator alike so softmax cancels it), stored fp8.
    Heads are software-pipelined: exp(h+3) overlaps av(h) (sT triple-buffered).
  - Key compaction: kernel() permutes each batch's tokens so unmasked keys
    come first; with <=640 unmasked keys the last 3 of 8 key chunks are
    entirely masked and skipped.  Runtime guard falls back otherwise.
  - AV: o[tq, 65] = sum_kt s^T[kt,tq].T @ v_ext[kt], DR over kt-chunk pairs
    (2 pairs + 1 plain fp8 tail); per-token softmax normalization is a
    per-partition scalar multiply.
  - o -> PE-transpose -> o^T fp8; proj via DR; residual add into x (fp32).
  - LN2 -> h2T bf16; a^T = gelu(W_fc^T @ h2T) feature-major; fc2 token-major;
    residual add; DMA out.  (MLP all bf16.)

Two program variants: `trivial=True` (unit LN gains, zero biases — the
distribution setup_inputs() generates) skips all bias/gain work; the general
variant applies them.  kernel() picks per call based on the actual inputs.
"""

import math

import numpy as np
import ml_dtypes

import concourse.bass as bass
import concourse.mybir as mybir
import concourse.tile as tile
from concourse import bacc
from concourse.bass import ds, ts
from concourse.masks import make_identity

F32 = mybir.dt.float32
BF16 = mybir.dt.bfloat16
FP8 = mybir.dt.float8e4
AF = mybir.ActivationFunctionType
ALU = mybir.AluOpType
DR = mybir.MatmulPerfMode.DoubleRow

T, C, NH, HD = 1024, 768, 12, 64
TT = T // 128          # 8 token tiles
CC = C // 128          # 6 feature chunks
FC = (4 * C) // 128    # 24 ffn-hidden chunks
N_CORES = 8
EPS = 1e-5
EXP_BIAS = -5.0 * math.log(2.0)   # exp output scaled 2^-5: fits fp8e4 range


def _bcast(ap_1d: bass.AP, p: int = 128) -> bass.AP:
    """Broadcast a 1-D DRAM AP across p partitions (zero partition stride)."""
    return bass.AP(tensor=ap_1d.tensor, offset=ap_1d.offset, ap=[[0, p]] + ap_1d.ap)


def build_bass(
    repeat: int = 1,
    trivial: bool = True,
    kt_chunks: int = 8,
    dr_qkv: bool = True,
    dr_scores: bool = True,
    dr_av: bool = True,
    dr_proj: bool = True,
    mlp_split: bool = False,
) -> bass.Bass:
    """kt_chunks: number of 128-key chunks attention processes (keys beyond
    kt_chunks*128 must be masked — kernel() permutes unmasked keys first and
    guards the count).  8 = full attention.

    dr_*: use fp8 DoubleRow for that stage's matmuls; stages without DR run
    in bf16 exactly like the original baseline (plain fp8 is SLOWER than
    bf16 on this hardware — measured ~1.5x — so never plain-fp8).
    dr_scores=False emits baseline-style [64,128] per-head slices, which
    require UNPERMUTED w_attn (kernel() permutes iff DR_SCORES)."""
    KT = kt_chunks
    qk_dt = FP8 if dr_scores else BF16    # scores operands
    sv_dt = FP8 if dr_av else BF16        # AV operands (sT, v_ext)
    exp_bias = EXP_BIAS if dr_av else None  # fp8 sT needs the 2^-5 range shift
    # Bacc (not plain Bass): its compile() runs generate_event_semaphores,
    # which splits multi-wait instructions — HW allows 1 wait per instruction.
    nc = bacc.Bacc(None)

    x_d = nc.dram_tensor("x", [T, C], F32, kind="ExternalInput")
    mask_d = nc.dram_tensor("mask01", [T], F32, kind="ExternalInput")
    # w_attn arrives PERMUTED (q/k columns regrouped for DoubleRow scores)
    wattn_d = nc.dram_tensor("w_attn", [C, 3 * C], FP8, kind="ExternalInput")
    wproj_d = nc.dram_tensor("w_proj", [C, C], FP8, kind="ExternalInput")
    if mlp_split:
        # hi = fp8(16*W), lo = fp8((16*W - hi)*16); out rescaled by 1/16 twice
        wfchi_d = nc.dram_tensor("w_fc_hi", [C, 4 * C], FP8, kind="ExternalInput")
        wfclo_d = nc.dram_tensor("w_fc_lo", [C, 4 * C], FP8, kind="ExternalInput")
        wfc2hi_d = nc.dram_tensor("w_fc2_hi", [4 * C, C], FP8, kind="ExternalInput")
        wfc2lo_d = nc.dram_tensor("w_fc2_lo", [4 * C, C], FP8, kind="ExternalInput")
    else:
        wfc_d = nc.dram_tensor("w_fc", [C, 4 * C], BF16, kind="ExternalInput")
        wfc2_d = nc.dram_tensor("w_fc2", [4 * C, C], BF16, kind="ExternalInput")
    if not trivial:
        ln1g_d = nc.dram_tensor("ln1_g", [C], F32, kind="ExternalInput")
        ln1b_d = nc.dram_tensor("ln1_b", [C], F32, kind="ExternalInput")
        ln2g_d = nc.dram_tensor("ln2_g", [C], F32, kind="ExternalInput")
        ln2b_d = nc.dram_tensor("ln2_b", [C], F32, kind="ExternalInput")
        battn_d = nc.dram_tensor("b_attn", [3 * C], F32, kind="ExternalInput")
        bproj_d = nc.dram_tensor("b_proj", [C], F32, kind="ExternalInput")
        bfc_d = nc.dram_tensor("b_fc", [4 * C], F32, kind="ExternalInput")
        bfc2_d = nc.dram_tensor("b_fc2", [C], F32, kind="ExternalInput")
    out_d = nc.dram_tensor("out", [T, C], F32, kind="ExternalOutput")

    with tile.TileContext(nc) as tc:
        from contextlib import ExitStack

        with ExitStack() as ctx:
            consts = ctx.enter_context(tc.tile_pool(name="consts", bufs=1))
            xpool = ctx.enter_context(tc.tile_pool(name="xpool", bufs=1))
            htmp_pool = ctx.enter_context(tc.tile_pool(name="htmp", bufs=3))
            stat_pool = ctx.enter_context(tc.tile_pool(name="stats", bufs=6))
            hT2_pool = ctx.enter_context(tc.tile_pool(name="hT2", bufs=1))
            wproj_pool = ctx.enter_context(tc.tile_pool(name="wproj", bufs=1))
            ps_mm = ctx.enter_context(tc.tile_pool(name="ps_mm", bufs=2, space="PSUM"))
            ps_tr = ctx.enter_context(tc.tile_pool(name="ps_tr", bufs=2, space="PSUM"))

            # ---------------- constants ----------------
            ident = consts.tile([128, 128], BF16, name="ident")
            make_identity(nc, ident)
            mask_col = consts.tile([128, TT], F32, name="mask_col")
            nc.gpsimd.dma_start(out=mask_col, in_=mask_d[:].rearrange("(t p) -> p t", p=128))
            eps_t = consts.tile([128, 1], F32, name="eps_t")
            nc.vector.memset(eps_t, EPS)
            expb_t = consts.tile([128, 1], F32, name="expb_t")
            nc.vector.memset(expb_t, EXP_BIAS)
            sixt_t = consts.tile([128, 1], F32, name="sixt_t")
            nc.vector.memset(sixt_t, 1.0 / 16.0)

            if not trivial:
                g1_bc = consts.tile([128, C], F32, name="g1_bc")
                b1_bc = consts.tile([128, C], F32, name="b1_bc")
                g2_bc = consts.tile([128, C], F32, name="g2_bc")
                b2_bc = consts.tile([128, C], F32, name="b2_bc")
                battnv_bc = consts.tile([128, C], F32, name="battnv_bc")
                bproj_bc = consts.tile([128, C], F32, name="bproj_bc")
                bfc2_bc = consts.tile([128, C], F32, name="bfc2_bc")
                nc.gpsimd.dma_start(out=g1_bc, in_=_bcast(ln1g_d[:]))
                nc.gpsimd.dma_start(out=b1_bc, in_=_bcast(ln1b_d[:]))
                nc.gpsimd.dma_start(out=g2_bc, in_=_bcast(ln2g_d[:]))
                nc.gpsimd.dma_start(out=b2_bc, in_=_bcast(ln2b_d[:]))
                nc.gpsimd.dma_start(out=battnv_bc, in_=_bcast(battn_d[ds(2 * C, C)]))
                nc.gpsimd.dma_start(out=bproj_bc, in_=_bcast(bproj_d[:]))
                nc.gpsimd.dma_start(out=bfc2_bc, in_=_bcast(bfc2_d[:]))
                # b_attn q/k biases arrive PERMUTED like the w_attn columns
                battn_qk = consts.tile([128, 12], F32, name="battn_qk")
                nc.gpsimd.dma_start(
                    out=battn_qk,
                    in_=battn_d[ds(0, 2 * C)].rearrange("(m p) -> p m", p=128),
                )
                bfc_col = consts.tile([128, FC], F32, name="bfc_col")
                nc.gpsimd.dma_start(
                    out=bfc_col, in_=bfc_d[:].rearrange("(m p) -> p m", p=128)
                )

            def layer_norm_to_hT(x_slice, g_bc, b_bc, hT, t, hT_lo=None):
                """LN over C (free dim) of one token tile; write transpose
                into hT[:, c, t*128:...] via PE transposes (copies on ScalarE).
                hT dtype (fp8 for h1T, bf16 for h2T) set by the copy cast."""
                stats = stat_pool.tile([128, 2, 6], F32, name="stats", tag="lnstats")
                for i in range(2):
                    nc.vector.bn_stats(out=stats[:, i, :], in_=x_slice[:, ts(i, 384)])
                mv = stat_pool.tile([128, 2], F32, name="mv", tag="lnmv")
                nc.vector.bn_aggr(out=mv, in_=stats)
                rstd = stat_pool.tile([128, 1], F32, name="rstd", tag="rstd")
                nc.scalar.activation(out=rstd, in_=mv[:, 1:2], func=AF.Sqrt, bias=eps_t[:, 0:1])
                nc.vector.reciprocal(rstd, rstd)
                hbf = htmp_pool.tile([128, C], BF16, name="hbf", tag="hbf")
                if trivial:
                    nc.vector.tensor_scalar(
                        out=hbf, in0=x_slice, scalar1=mv[:, 0:1], scalar2=rstd,
                        op0=ALU.subtract, op1=ALU.mult,
                    )
                else:
                    htmp = htmp_pool.tile([128, C], F32, name="htmp", tag="htmp")
                    nc.vector.tensor_scalar(
                        out=htmp, in0=x_slice, scalar1=mv[:, 0:1], scalar2=rstd,
                        op0=ALU.subtract, op1=ALU.mult,
                    )
                    nc.vector.tensor_mul(htmp, htmp, g_bc)
                    nc.vector.tensor_add(hbf, htmp, b_bc)
                for c in range(CC):
                    ptr = ps_tr.tile([128, 128], BF16, name="ptr", tag="tr")
                    nc.tensor.transpose(ptr, hbf[:, ts(c, 128)], ident)
                    nc.scalar.copy(hT[:, c, ts(t, 128)], ptr)
                    if hT_lo is not None:
                        # subnormal-fp8 residual of the fp8 cast (no rescale)
                        nc.vector.tensor_sub(
                            hT_lo[:, c, ts(t, 128)], ptr, hT[:, c, ts(t, 128)]
                        )

            # ================= one full block (repeatable) =================
            for _rep in range(repeat):
              # residual stream, token-major fp32 (per-tile DMAs so LN1 can
              # start as soon as the first rows land)
              x_t = xpool.tile([128, TT, C], F32, name="x_t", tag="x_t")
              for t in range(TT):
                  nc.sync.dma_start(out=x_t[:, t, :], in_=x_d[ts(t, 128), :])

              # ---- attention scope ----
              with ExitStack() as actx:
                wattn_pool = actx.enter_context(tc.tile_pool(name="wattn", bufs=1))
                ps_s = actx.enter_context(tc.tile_pool(name="ps_s", bufs=2, space="PSUM"))
                hT_pool = actx.enter_context(tc.tile_pool(name="hT1", bufs=1))
                qkT_pool = actx.enter_context(tc.tile_pool(name="qkT", bufs=1))
                vext_pool = actx.enter_context(tc.tile_pool(name="vext", bufs=1))
                sT_pool = actx.enter_context(tc.tile_pool(name="sT", bufs=3))
                opool = actx.enter_context(tc.tile_pool(name="opool", bufs=1))

                wattn_sb = wattn_pool.tile([128, CC, 3 * C], FP8, name="wattn_sb")
                nc.gpsimd.dma_start(
                    out=wattn_sb, in_=wattn_d[:, :].rearrange("(c p) n -> p c n", p=128)
                )
                wproj_sb = wproj_pool.tile([128, CC, C], FP8, name="wproj_sb")
                nc.gpsimd.dma_start(
                    out=wproj_sb, in_=wproj_d[:, :].rearrange("(c p) n -> p c n", p=128)
                )

                h1T = hT_pool.tile([128, CC, T], FP8, name="h1T", tag="hT")
                qT = qkT_pool.tile([128, CC, T], qk_dt, name="qT", tag="qT")
                kT = qkT_pool.tile([128, CC, KT * 128], qk_dt, name="kT", tag="kT")
                vext = vext_pool.tile([128, NH, KT, HD + 1], sv_dt, name="vext")
                nc.vector.memset(vext[:, :, :, HD : HD + 1], 1.0)
                if trivial:
                    # mask the ones column up front (v columns get the mask
                    # folded into their psum->sbuf copies below)
                    for t in range(KT):
                        nc.vector.tensor_scalar_mul(
                            out=vext[:, :, t, HD : HD + 1],
                            in0=vext[:, :, t, HD : HD + 1],
                            scalar1=mask_col[:, t : t + 1],
                        )

                # q^T / k^T feature-major, emitted token-half by token-half
                # so the first half's matmuls run while LN1 of tiles 4-7 is
                # still computing (the PE stream is in-order)
                def qk_phase(nqi):
                    for m in range(12):
                        dest = qT if m < 6 else kT
                        nlim = T if m < 6 else KT * 128
                        n0 = nqi * 512
                        nsz = min(512, nlim - n0)
                        if nsz <= 0:
                            continue
                        pq = ps_mm.tile([128, nsz], F32, name="pq", tag="mm")
                        if dr_qkv:
                            for c in range(0, CC, 2):
                                nc.tensor.matmul(
                                    pq,
                                    lhsT=wattn_sb[:, c : c + 2, ts(m, 128)],
                                    rhs=h1T[:, c : c + 2, ds(n0, nsz)],
                                    start=(c == 0),
                                    stop=(c == CC - 2),
                                    perf_mode=DR,
                                )
                        else:
                            for c in range(CC):
                                nc.tensor.matmul(
                                    pq,
                                    lhsT=wattn_sb[:, c, ts(m, 128)],
                                    rhs=h1T[:, c, ds(n0, nsz)],
                                    start=(c == 0),
                                    stop=(c == CC - 1),
                                )
                        if trivial:
                            nc.vector.tensor_copy(dest[:, m % 6, ds(n0, nsz)], pq)
                        else:
                            nc.scalar.activation(
                                out=dest[:, m % 6, ds(n0, nsz)], in_=pq,
                                func=AF.Identity, bias=battn_qk[:, m : m + 1],
                            )

                for t in range(4):
                    layer_norm_to_hT(
                        x_t[:, t, :],
                        None if trivial else g1_bc,
                        None if trivial else b1_bc,
                        h1T, t,
                    )
                qk_phase(0)
                for t in range(4, TT):
                    layer_norm_to_hT(
                        x_t[:, t, :],
                        None if trivial else g1_bc,
                        None if trivial else b1_bc,
                        h1T, t,
                    )
                qk_phase(1)

                sT_tiles = {}

                def emit_scores(h):
                    # scores^T for head h: DoubleRow over the head dim (64 =
                    # 32 partitions x 2 chunk-slots, from the host-side W_attn
                    # column permutation); one big [128, 2048] exp per pair of
                    # kt chunks (amortizes the ACT op overhead)
                    sT = sT_pool.tile([128, KT, T], sv_dt, name="sT", tag="sT")
                    sT_tiles[h] = sT
                    for kt in range(KT):
                        pss = ps_s.tile([128, 2, 512], F32, name="pss", tag="ss")
                        for nq in range(2):
                            if dr_scores:
                                g, i = divmod(h, 4)
                                r0 = 32 * i
                                nc.tensor.matmul(
                                    pss[:, nq, :],
                                    lhsT=kT[ds(r0, 32), ds(2 * g, 2), ts(kt, 128)],
                                    rhs=qT[ds(r0, 32), ds(2 * g, 2), ts(nq, 512)],
                                    start=True,
                                    stop=True,
                                    perf_mode=DR,
                                    # base_partition() rejects 96; rows<=32 may
                                    # sit at any quadrant via explicit position
                                    tile_position=(r0, 0),
                                )
                            else:
                                hc, hr = divmod(h, 2)
                                r0 = hr * 64
                                nc.tensor.matmul(
                                    pss[:, nq, :],
                                    lhsT=kT[ds(r0, 64), hc, ts(kt, 128)],
                                    rhs=qT[ds(r0, 64), hc, ts(nq, 512)],
                                    start=True,
                                    stop=True,
                                )
                        nc.scalar.activation(
                            out=sT[:, kt, :],
                            in_=pss.rearrange("p a b -> p (a b)"),
                            func=AF.Exp,
                            scale=0.125,
                            bias=0.0 if exp_bias is None else expb_t[:, 0:1],
                        )

                # prime the head pipeline (sT is triple-buffered)
                emit_scores(0)
                emit_scores(1)
                emit_scores(2)

                # v token-major, scattered per head into v_ext (batched copies)
                for t in range(KT):
                    for n0, nsz in ((0, 512), (512, 256)):
                        pv = ps_mm.tile([128, nsz], F32, name="pv", tag="mm")
                        if dr_qkv:
                            for c in range(0, CC, 2):
                                nc.tensor.matmul(
                                    pv,
                                    lhsT=h1T[:, c : c + 2, ts(t, 128)],
                                    rhs=wattn_sb[:, c : c + 2, ds(2 * C + n0, nsz)],
                                    start=(c == 0),
                                    stop=(c == CC - 2),
                                    perf_mode=DR,
                                )
                        else:
                            for c in range(CC):
                                nc.tensor.matmul(
                                    pv,
                                    lhsT=h1T[:, c, ts(t, 128)],
                                    rhs=wattn_sb[:, c, ds(2 * C + n0, nsz)],
                                    start=(c == 0),
                                    stop=(c == CC - 1),
                                )
                        h0, h1 = n0 // HD, (n0 + nsz) // HD
                        pv_h = pv.rearrange("p (h d) -> p h d", d=HD)
                        if trivial:
                            # mask folded into the psum->sbuf copy
                            nc.vector.tensor_scalar_mul(
                                out=vext[:, h0:h1, t, 0:HD], in0=pv_h,
                                scalar1=mask_col[:, t : t + 1],
                            )
                        else:
                            nc.vector.tensor_add(
                                out=vext[:, h0:h1, t, 0:HD], in0=pv_h,
                                in1=battnv_bc[:, ds(n0, nsz)].rearrange(
                                    "p (h d) -> p h d", d=HD
                                ),
                            )
                if not trivial:
                    # fold the key-padding mask into v_ext (incl. ones col)
                    for t in range(KT):
                        nc.vector.tensor_scalar_mul(
                            out=vext[:, :, t, :], in0=vext[:, :, t, :],
                            scalar1=mask_col[:, t : t + 1],
                        )

                o_t = opool.tile([128, TT, C], BF16, name="o_t", tag="op")
                oT = qkT_pool.tile([128, CC, T], FP8, name="oT", tag="oT")
                KT_PAIRS = KT // 2
                for h in range(NH):
                    sT = sT_tiles.pop(h)
                    for tq in range(TT):
                        pav = ps_mm.tile([128, HD + 1], F32, name="pav", tag="mm")
                        if dr_av:
                            for j in range(KT_PAIRS):
                                nc.tensor.matmul(
                                    pav,
                                    lhsT=sT[:, 2 * j : 2 * j + 2, ts(tq, 128)],
                                    rhs=vext[:, h, 2 * j : 2 * j + 2, :],
                                    start=(j == 0),
                                    stop=(KT % 2 == 0 and j == KT_PAIRS - 1),
                                    perf_mode=DR,
                                )
                            if KT % 2 == 1:
                                nc.tensor.matmul(
                                    pav,
                                    lhsT=sT[:, KT - 1, ts(tq, 128)],
                                    rhs=vext[:, h, KT - 1, :],
                                    start=False,
                                    stop=True,
                                )
                        else:
                            for kt in range(KT):
                                nc.tensor.matmul(
                                    pav,
                                    lhsT=sT[:, kt, ts(tq, 128)],
                                    rhs=vext[:, h, kt, :],
                                    start=(kt == 0),
                                    stop=(kt == KT - 1),
                                )
                        rec = stat_pool.tile([128, 1], F32, name="rec", tag="rec")
                        nc.vector.reciprocal(rec, pav[:, HD : HD + 1])
                        nc.vector.tensor_scalar_mul(
                            out=o_t[:, tq, ts(h, HD)], in0=pav[:, 0:HD], scalar1=rec
                        )
                    if h + 3 < NH:
                        emit_scores(h + 3)
                    if h % 2 == 1:
                        # heads 2c,2c+1 done -> transpose o chunk c now (PE has
                        # slack while ACT exps; copies on DVE, not the busy ACT)
                        c = h // 2
                        for t in range(TT):
                            ptr = ps_tr.tile([128, 128], BF16, name="ptr2", tag="tr")
                            nc.tensor.transpose(ptr, o_t[:, t, ts(c, 128)], ident)
                            nc.vector.tensor_copy(oT[:, c, ts(t, 128)], ptr)

                # proj + residual into x (fp32); LN2 + h2T per 4-tile group
                # so fcT's first half (needs h2T tiles 0-3) starts while the
                # second half of proj still runs
                h2_dt = FP8 if mlp_split else BF16
                h2T = hT2_pool.tile([128, CC, T], h2_dt, name="h2T", tag="hT2")
                h2T_lo = (
                    hT2_pool.tile([128, CC, T], FP8, name="h2Tlo", tag="hT2lo")
                    if mlp_split else None
                )
                for grp in range(2):
                    for t in range(grp * 4, grp * 4 + 4):
                        for n0, nsz in ((0, 512), (512, 256)):
                            pp = ps_mm.tile([128, nsz], F32, name="pp", tag="mm")
                            if dr_proj:
                                for c in range(0, CC, 2):
                                    nc.tensor.matmul(
                                        pp,
                                        lhsT=oT[:, c : c + 2, ts(t, 128)],
                                        rhs=wproj_sb[:, c : c + 2, ds(n0, nsz)],
                                        start=(c == 0),
                                        stop=(c == CC - 2),
                                        perf_mode=DR,
                                    )
                            else:
                                for c in range(CC):
                                    nc.tensor.matmul(
                                        pp,
                                        lhsT=oT[:, c, ts(t, 128)],
                                        rhs=wproj_sb[:, c, ds(n0, nsz)],
                                        start=(c == 0),
                                        stop=(c == CC - 1),
                                    )
                            if not trivial:
                                nc.vector.tensor_add(pp, pp, bproj_bc[:, ds(n0, nsz)])
                            nc.vector.tensor_add(
                                x_t[:, t, ds(n0, nsz)], x_t[:, t, ds(n0, nsz)], pp
                            )
                    for t in range(grp * 4, grp * 4 + 4):
                        layer_norm_to_hT(
                            x_t[:, t, :],
                            None if trivial else g2_bc,
                            None if trivial else b2_bc,
                            h2T, t, hT_lo=h2T_lo,
                        )

              # ---- MLP scope ----
              with ExitStack() as mctx:
                aT_pool = mctx.enter_context(tc.tile_pool(name="aT", bufs=1))
                outsb_pool = mctx.enter_context(tc.tile_pool(name="outsb", bufs=2))

                if not mlp_split:
                    wfc_pool = mctx.enter_context(tc.tile_pool(name="wfc", bufs=1))
                    wfc2_pool = mctx.enter_context(tc.tile_pool(name="wfc2", bufs=1))
                    ps_mlp = mctx.enter_context(
                        tc.tile_pool(name="ps_mlp", bufs=4, space="PSUM")
                    )
                    wfc_sb = wfc_pool.tile([128, CC, 4 * C], BF16, name="wfc_sb")
                    nc.gpsimd.dma_start(
                        out=wfc_sb, in_=wfc_d[:, :].rearrange("(c p) n -> p c n", p=128)
                    )
                    wfc2_sb = wfc2_pool.tile([128, FC, C], BF16, name="wfc2_sb")
                    nc.gpsimd.dma_start(
                        out=wfc2_sb, in_=wfc2_d[:, :].rearrange("(m p) n -> p m n", p=128)
                    )

                    for half in range(2):
                        aT = aT_pool.tile([128, FC, 512], BF16, name="aT", tag="aT")
                        for m in range(FC):
                            pf = ps_mlp.tile([128, 512], F32, name="pf", tag="mmp")
                            for c in range(CC):
                                nc.tensor.matmul(
                                    pf,
                                    lhsT=wfc_sb[:, c, ts(m, 128)],
                                    rhs=h2T[:, c, ds(half * 512, 512)],
                                    start=(c == 0),
                                    stop=(c == CC - 1),
                                )
                            nc.scalar.activation(
                                out=aT[:, m, :], in_=pf, func=AF.Gelu_apprx_tanh,
                                bias=0.0 if trivial else bfc_col[:, m : m + 1],
                            )
                        for i in range(4):
                            t = half * 4 + i
                            outsb = outsb_pool.tile([128, C], F32, name="outsb", tag="outsb")
                            for n0, nsz in ((0, 512), (512, 256)):
                                pf2 = ps_mlp.tile([128, nsz], F32, name="pf2", tag="mmp")
                                for m in range(FC):
                                    nc.tensor.matmul(
                                        pf2,
                                        lhsT=aT[:, m, ts(i, 128)],
                                        rhs=wfc2_sb[:, m, ds(n0, nsz)],
                                        start=(m == 0),
                                        stop=(m == FC - 1),
                                    )
                                if not trivial:
                                    nc.vector.tensor_add(pf2, pf2, bfc2_bc[:, ds(n0, nsz)])
                                nc.vector.tensor_add(
                                    outsb[:, ds(n0, nsz)], x_t[:, t, ds(n0, nsz)], pf2
                                )
                            nc.sync.dma_start(out=out_d[ts(t, 128), :], in_=outsb)
                else:
                    # split-fp8 MLP: W stored as hi=fp8(16W), lo=fp8((16W-hi)*16).
                    # P1 accumulates scale-16 terms (hi@hi and act_lo@hi: act lo
                    # is the UNSCALED subnormal-fp8 cast residual), P2 the
                    # scale-256 term (hi@lo); combined = P1 + P2/16, and the
                    # final 1/16 folds into the gelu scale / output rescale.
                    wfc_pool = mctx.enter_context(tc.tile_pool(name="wfc", bufs=1))
                    wfc2_pool = mctx.enter_context(tc.tile_pool(name="wfc2", bufs=1))
                    a8_pool = mctx.enter_context(tc.tile_pool(name="a8", bufs=1))
                    ps_mlp = mctx.enter_context(
                        tc.tile_pool(name="ps_mlp", bufs=4, space="PSUM")
                    )
                    wfchi_sb = wfc_pool.tile([128, CC, 4 * C], FP8, name="wfchi_sb")
                    wfclo_sb = wfc_pool.tile([128, CC, 4 * C], FP8, name="wfclo_sb")
                    nc.gpsimd.dma_start(
                        out=wfchi_sb, in_=wfchi_d[:, :].rearrange("(c p) n -> p c n", p=128)
                    )
                    nc.gpsimd.dma_start(
                        out=wfclo_sb, in_=wfclo_d[:, :].rearrange("(c p) n -> p c n", p=128)
                    )
                    wfc2hi_sb = wfc2_pool.tile([128, FC, C], FP8, name="wfc2hi_sb")
                    wfc2lo_sb = wfc2_pool.tile([128, FC, C], FP8, name="wfc2lo_sb")
                    nc.gpsimd.dma_start(
                        out=wfc2hi_sb, in_=wfc2hi_d[:, :].rearrange("(m p) n -> p m n", p=128)
                    )
                    nc.gpsimd.dma_start(
                        out=wfc2lo_sb, in_=wfc2lo_d[:, :].rearrange("(m p) n -> p m n", p=128)
                    )

                    for half in range(2):
                        aT = aT_pool.tile([128, FC, 512], BF16, name="aT", tag="aT")
                        ahi = a8_pool.tile([128, FC, 512], FP8, name="ahi", tag="ahi")
                        alo = a8_pool.tile([128, FC, 512], FP8, name="alo", tag="alo")
                        hcols = ds(half * 512, 512)
                        for m in range(FC):
                            p1 = ps_mlp.tile([128, 512], F32, name="p1", tag="mmp")
                            for li, (wsb, hT_) in enumerate(
                                ((wfchi_sb, h2T), (wfclo_sb, h2T), (wfchi_sb, h2T_lo))
                            ):
                                for c in range(0, CC, 2):
                                    nc.tensor.matmul(
                                        p1,
                                        lhsT=wsb[:, c : c + 2, ts(m, 128)],
                                        rhs=hT_[:, c : c + 2, hcols],
                                        start=(li == 0 and c == 0),
                                        stop=(li == 2 and c == CC - 2),
                                        perf_mode=DR,
                                    )
                            nc.scalar.activation(
                                out=aT[:, m, :], in_=p1, func=AF.Gelu_apprx_tanh,
                                scale=1.0 / 16.0,
                                bias=0.0 if trivial else bfc_col[:, m : m + 1],
                            )
                            nc.vector.tensor_copy(ahi[:, m, :], aT[:, m, :])
                            nc.vector.tensor_sub(alo[:, m, :], aT[:, m, :], ahi[:, m, :])
                        for i in range(4):
                            t = half * 4 + i
                            outsb = outsb_pool.tile([128, C], F32, name="outsb", tag="outsb")
                            for n0, nsz in ((0, 512), (512, 256)):
                                q1 = ps_mlp.tile([128, nsz], F32, name="q1", tag="mmp")
                                for li, (aT_, wsb) in enumerate(
                                    ((ahi, wfc2hi_sb), (ahi, wfc2lo_sb), (alo, wfc2hi_sb))
                                ):
                                    for m in range(0, FC, 2):
                                        nc.tensor.matmul(
                                            q1,
                                            lhsT=aT_[:, m : m + 2, ts(i, 128)],
                                            rhs=wsb[:, m : m + 2, ds(n0, nsz)],
                                            start=(li == 0 and m == 0),
                                            stop=(li == 2 and m == FC - 2),
                                            perf_mode=DR,
                                        )
                                nc.vector.tensor_scalar_mul(
                                    outsb[:, ds(n0, nsz)], q1, sixt_t[:, 0:1]
                                )
                                nc.vector.tensor_add(
                                    outsb[:, ds(n0, nsz)],
                                    outsb[:, ds(n0, nsz)],
                                    x_t[:, t, ds(n0, nsz)],
                                )
                                if not trivial:
                                    nc.vector.tensor_add(
                                        outsb[:, ds(n0, nsz)],
                                        outsb[:, ds(n0, nsz)],
                                        bfc2_bc[:, ds(n0, nsz)],
                                    )
                            nc.sync.dma_start(out=out_d[ts(t, 128), :], in_=outsb)

    return nc


_NC_CACHE = {}

COMPACT_KT = 5  # attention processes 5*128 = 640 keys; guarded in kernel()

# Graded configuration: DoubleRow only where hardware-verified faster.
DR_QKV = True
DR_SCORES = False
DR_AV = False
DR_PROJ = True
MLP_SPLIT = False


def _get_nc(trivial: bool = True, kt_chunks: int = COMPACT_KT) -> bass.Bass:
    key = (trivial, kt_chunks)
    if key not in _NC_CACHE:
        nc = build_bass(
            trivial=trivial, kt_chunks=kt_chunks,
            dr_qkv=DR_QKV, dr_scores=DR_SCORES, dr_av=DR_AV, dr_proj=DR_PROJ,
            mlp_split=MLP_SPLIT,
        )
        nc.finalize()
        _NC_CACHE[key] = nc
    return _NC_CACHE[key]


TRACE = False
LAST_RESULTS = None
LAST_IN_MAPS = None


def _permute_qk_cols(w_qk: np.ndarray) -> np.ndarray:
    """Reorder q-or-k columns [.., 768] so head h's dims 0-31 / 32-63 land in
    feature chunks 2*(h//4) / 2*(h//4)+1 at partitions 32*(h%4)."""
    perm = np.empty(C, np.int64)
    pos = 0
    for g in range(3):           # head groups of 4
        for half in range(2):    # dims 0-31 | 32-63
            for i in range(4):   # head within group
                h = 4 * g + i
                perm[pos : pos + 32] = h * HD + half * 32 + np.arange(32)
                pos += 32
    return w_qk[..., perm]


def kernel(**inputs) -> np.ndarray:
    global LAST_RESULTS, LAST_IN_MAPS

    f32 = lambda a: np.ascontiguousarray(np.asarray(a, dtype=np.float32))
    bf = lambda a: np.ascontiguousarray(
        np.asarray(a, dtype=np.float32).astype(ml_dtypes.bfloat16)
    )
    f8 = lambda a: np.ascontiguousarray(
        np.asarray(a, dtype=np.float32).astype(ml_dtypes.float8_e4m3)
    )

    x = f32(inputs["x"])                       # [8, 1024, 768]
    mask = np.asarray(inputs["attn_mask"])     # [8, 1024] int32

    lng1, lnb1 = f32(inputs["ln1_g"]), f32(inputs["ln1_b"])
    lng2, lnb2 = f32(inputs["ln2_g"]), f32(inputs["ln2_b"])
    ba, bp = f32(inputs["b_attn"]), f32(inputs["b_proj"])
    bf_, bf2 = f32(inputs["b_fc"]), f32(inputs["b_fc2"])
    trivial = bool(
        (lng1 == 1).all() and (lnb1 == 0).all() and (lng2 == 1).all()
        and (lnb2 == 0).all() and (ba == 0).all() and (bp == 0).all()
        and (bf_ == 0).all() and (bf2 == 0).all()
    )

    # Key compaction: permute tokens per batch so unmasked keys come first.
    # Attention is permutation-equivariant over keys, and LN/MLP/residual are
    # per-token, so permuting rows of x and un-permuting the output is exact.
    # With <= COMPACT_KT*128 unmasked keys the remaining key chunks are all
    # masked (zero contribution) and can be skipped entirely.
    mask01 = (mask != 0)
    counts = mask01.sum(axis=1)
    compact = bool(counts.max() <= COMPACT_KT * 128)
    kt_chunks = COMPACT_KT if compact else TT

    perms = []
    for b in range(N_CORES):
        perm = np.argsort(~mask01[b], kind="stable")  # unmasked first
        perms.append(perm)

    nc = _get_nc(trivial, kt_chunks)

    # q/k column permutation for DoubleRow scores (see _permute_qk_cols)
    W_attn = f32(inputs["W_attn"])
    if DR_SCORES:
        W_attn_perm = np.concatenate(
            [
                _permute_qk_cols(W_attn[:, 0:C]),
                _permute_qk_cols(W_attn[:, C : 2 * C]),
                W_attn[:, 2 * C :],
            ],
            axis=1,
        )
    else:
        W_attn_perm = W_attn
    common = {
        "w_attn": f8(W_attn_perm),
        "w_proj": f8(inputs["W_proj"]),
    }
    if MLP_SPLIT:
        def _wsplit(w):
            ws = f32(w) * 16.0
            hi = np.asarray(ws, np.float32).astype(ml_dtypes.float8_e4m3)
            lo = ws - hi.astype(np.float32)   # raw residual (subnormal fp8)
            return np.ascontiguousarray(hi), np.ascontiguousarray(
                lo.astype(ml_dtypes.float8_e4m3)
            )
        common["w_fc_hi"], common["w_fc_lo"] = _wsplit(inputs["W_fc"])
        common["w_fc2_hi"], common["w_fc2_lo"] = _wsplit(inputs["W_fc2"])
    else:
        common["w_fc"] = bf(inputs["W_fc"])
        common["w_fc2"] = bf(inputs["W_fc2"])
    if not trivial:
        if DR_SCORES:
            ba_perm = np.concatenate(
                [_permute_qk_cols(ba[0:C]), _permute_qk_cols(ba[C : 2 * C]), ba[2 * C :]]
            )
        else:
            ba_perm = ba
        common.update(
            ln1_g=lng1, ln1_b=lnb1, ln2_g=lng2, ln2_b=lnb2,
            b_attn=ba_perm, b_proj=bp, b_fc=bf_, b_fc2=bf2,
        )
    in_maps = []
    for b in range(N_CORES):
        m = dict(common)
        m["x"] = np.ascontiguousarray(x[b][perms[b]])
        m["mask01"] = np.ascontiguousarray(mask01[b][perms[b]].astype(np.float32))
        in_maps.append(m)

    from concourse.bass_utils import run_bass_kernel_spmd

    LAST_IN_MAPS = in_maps
    res = run_bass_kernel_spmd(nc, in_maps, core_ids=list(range(N_CORES)), trace=TRACE)
    LAST_RESULTS = res
    out = np.empty((N_CORES, T, C), np.float32)
    for b in range(N_CORES):
        out[b, perms[b]] = np.asarray(res.results[b]["out"])
    return out


# revision 22
# speedup vs baseline: 1.6707x; 1.2726x over previous
"""Trainium2 Bass kernel for one GPT-style transformer block.

Problem: B=8, T=1024, C=768, NH=12 heads (HD=64), pre-LN attention + MLP,
key-padding mask, tanh-gelu.  Sharding: data-parallel over batch — each of
the 8 NeuronCores processes one batch element end-to-end (no collectives).

Per-core dataflow:
  - Attention matmuls run in fp8-e4m3 with DoubleRow perf mode (two 128-row
    k-tiles contracted per instruction, 2x PE throughput vs bf16); the MLP
    stays bf16 (fp8 there would blow the 2e-2 error budget; attention's
    contribution to the output is small so its fp8 noise is damped).
  - x resident token-major fp32 [128p, 8t, 768] (residual stream)
  - LN1 on token-major tiles -> bf16 -> PE-transpose -> h1T fp8 feature-major
  - q^T,k^T computed feature-major via DR matmuls (lhsT=W_attn chunk pairs,
    rhs=h1T chunk pairs).  W_attn's q/k columns are PERMUTED host-side so
    that head h's dims 0-31 and 32-63 land in adjacent feature chunks at
    partitions 32*(h%4): a [32p, 2, N] slice of qT/kT is then a legal
    DoubleRow operand pair contracting the full head dim (64).
  - v computed token-major (DR over feature-chunk pairs), stored per-head as
    v_ext fp8 [128p, head, kt, 65] with a ones-column (col 64) so the
    attention row-sum (softmax denominator) falls out of the same matmul.
    The key-padding mask is folded in by zeroing masked v_ext rows entirely.
  - scores TRANSPOSED per head: s^T[k, q] via DR (lhsT = kT [32,2,128],
    rhs = qT [32,2,512]); exp(s/8)*2^-5 on the scalar engine (bias -5ln2
    keeps the heavy-tailed exp inside fp8 range, max ~178 < 240; the 2^-5
    hits numerator and denominator alike so softmax cancels it), stored fp8.
    Heads are software-pipelined: exp(h+3) overlaps av(h) (sT triple-buffered).
  - Key compaction: kernel() permutes each batch's tokens so unmasked keys
    come first; with <=640 unmasked keys the last 3 of 8 key chunks are
    entirely masked and skipped.  Runtime guard falls back otherwise.
  - AV: o[tq, 65] = sum_kt s^T[kt,tq].T @ v_ext[kt], DR over kt-chunk pairs
    (2 pairs + 1 plain fp8 tail); per-token softmax normalization is a
    per-partition scalar multiply.
  - o -> PE-transpose -> o^T fp8; proj via DR; residual add into x (fp32).
  - LN2 -> h2T bf16; a^T = gelu(W_fc^T @ h2T) feature-major; fc2 token-major;
    residual add; DMA out.  (MLP all bf16.)

Two program variants: `trivial=True` (unit LN gains, zero biases — the
distribution setup_inputs() generates) skips all bias/gain work; the general
variant applies them.  kernel() picks per call based on the actual inputs.
"""

import math

import numpy as np
import ml_dtypes

import concourse.bass as bass
import concourse.mybir as mybir
import concourse.tile as tile
from concourse import bacc
from concourse.bass import ds, ts
from concourse.masks import make_identity

F32 = mybir.dt.float32
BF16 = mybir.dt.bfloat16
FP8 = mybir.dt.float8e4
AF = mybir.ActivationFunctionType
ALU = mybir.AluOpType
DR = mybir.MatmulPerfMode.DoubleRow

T, C, NH, HD = 1024, 768, 12, 64
TT = T // 128          # 8 token tiles
CC = C // 128          # 6 feature chunks
FC = (4 * C) // 128    # 24 ffn-hidden chunks
N_CORES = 8
EPS = 1e-5
EXP_BIAS = -5.0 * math.log(2.0)   # exp output scaled 2^-5: fits fp8e4 range


def _bcast(ap_1d: bass.AP, p: int = 128) -> bass.AP:
    """Broadcast a 1-D DRAM AP across p partitions (zero partition stride)."""
    return bass.AP(tensor=ap_1d.tensor, offset=ap_1d.offset, ap=[[0, p]] + ap_1d.ap)


def build_bass(
    repeat: int = 1,
    trivial: bool = True,
    kt_chunks: int = 8,
    dr_qkv: bool = True,
    dr_scores: bool = True,
    dr_av: bool = True,
    dr_proj: bool = True,
    mlp_split: bool = False,
) -> bass.Bass:
    """kt_chunks: number of 128-key chunks attention processes (keys beyond
    kt_chunks*128 must be masked — kernel() permutes unmasked keys first and
    guards the count).  8 = full attention.

    dr_*: use fp8 DoubleRow for that stage's matmuls; stages without DR run
    in bf16 exactly like the original baseline (plain fp8 is SLOWER than
    bf16 on this hardware — measured ~1.5x — so never plain-fp8).
    dr_scores=False emits baseline-style [64,128] per-head slices, which
    require UNPERMUTED w_attn (kernel() permutes iff DR_SCORES)."""
    KT = kt_chunks
    qk_dt = FP8 if dr_scores else BF16    # scores operands
    sv_dt = FP8 if dr_av else BF16        # AV operands (sT, v_ext)
    exp_bias = EXP_BIAS if dr_av else None  # fp8 sT needs the 2^-5 range shift
    # Bacc (not plain Bass): its compile() runs generate_event_semaphores,
    # which splits multi-wait instructions — HW allows 1 wait per instruction.
    nc = bacc.Bacc(None)

    x_d = nc.dram_tensor("x", [T, C], F32, kind="ExternalInput")
    mask_d = nc.dram_tensor("mask01", [T], F32, kind="ExternalInput")
    # w_attn arrives PERMUTED (q/k columns regrouped for DoubleRow scores)
    wattn_d = nc.dram_tensor("w_attn", [C, 3 * C], FP8, kind="ExternalInput")
    wproj_d = nc.dram_tensor("w_proj", [C, C], FP8, kind="ExternalInput")
    if mlp_split:
        # hi = fp8(16*W), lo = fp8((16*W - hi)*16); out rescaled by 1/16 twice
        wfchi_d = nc.dram_tensor("w_fc_hi", [C, 4 * C], FP8, kind="ExternalInput")
        wfclo_d = nc.dram_tensor("w_fc_lo", [C, 4 * C], FP8, kind="ExternalInput")
        wfc2hi_d = nc.dram_tensor("w_fc2_hi", [4 * C, C], FP8, kind="ExternalInput")
        wfc2lo_d = nc.dram_tensor("w_fc2_lo", [4 * C, C], FP8, kind="ExternalInput")
    else:
        wfc_d = nc.dram_tensor("w_fc", [C, 4 * C], BF16, kind="ExternalInput")
        wfc2_d = nc.dram_tensor("w_fc2", [4 * C, C], BF16, kind="ExternalInput")
    if not trivial:
        ln1g_d = nc.dram_tensor("ln1_g", [C], F32, kind="ExternalInput")
        ln1b_d = nc.dram_tensor("ln1_b", [C], F32, kind="ExternalInput")
        ln2g_d = nc.dram_tensor("ln2_g", [C], F32, kind="ExternalInput")
        ln2b_d = nc.dram_tensor("ln2_b", [C], F32, kind="ExternalInput")
        battn_d = nc.dram_tensor("b_attn", [3 * C], F32, kind="ExternalInput")
        bproj_d = nc.dram_tensor("b_proj", [C], F32, kind="ExternalInput")
        bfc_d = nc.dram_tensor("b_fc", [4 * C], F32, kind="ExternalInput")
        bfc2_d = nc.dram_tensor("b_fc2", [C], F32, kind="ExternalInput")
    out_d = nc.dram_tensor("out", [T, C], F32, kind="ExternalOutput")

    with tile.TileContext(nc) as tc:
        from contextlib import ExitStack

        with ExitStack() as ctx:
            consts = ctx.enter_context(tc.tile_pool(name="consts", bufs=1))
            xpool = ctx.enter_context(tc.tile_pool(name="xpool", bufs=1))
            htmp_pool = ctx.enter_context(tc.tile_pool(name="htmp", bufs=3))
            stat_pool = ctx.enter_context(tc.tile_pool(name="stats", bufs=6))
            hT2_pool = ctx.enter_context(tc.tile_pool(name="hT2", bufs=1))
            wproj_pool = ctx.enter_context(tc.tile_pool(name="wproj", bufs=1))
            ps_mm = ctx.enter_context(tc.tile_pool(name="ps_mm", bufs=2, space="PSUM"))
            ps_tr = ctx.enter_context(tc.tile_pool(name="ps_tr", bufs=2, space="PSUM"))
            if not mlp_split:
                ps_s = ctx.enter_context(tc.tile_pool(name="ps_s", bufs=2, space="PSUM"))

            # ---------------- constants ----------------
            ident = consts.tile([128, 128], BF16, name="ident")
            make_identity(nc, ident)
            mask_col = consts.tile([128, TT], F32, name="mask_col")
            nc.gpsimd.dma_start(out=mask_col, in_=mask_d[:].rearrange("(t p) -> p t", p=128))
            eps_t = consts.tile([128, 1], F32, name="eps_t")
            nc.vector.memset(eps_t, EPS)
            expb_t = consts.tile([128, 1], F32, name="expb_t")
            nc.vector.memset(expb_t, EXP_BIAS)
            sixt_t = consts.tile([128, 1], F32, name="sixt_t")
            nc.vector.memset(sixt_t, 1.0 / 16.0)

            if not trivial:
                g1_bc = consts.tile([128, C], F32, name="g1_bc")
                b1_bc = consts.tile([128, C], F32, name="b1_bc")
                g2_bc = consts.tile([128, C], F32, name="g2_bc")
                b2_bc = consts.tile([128, C], F32, name="b2_bc")
                battnv_bc = consts.tile([128, C], F32, name="battnv_bc")
                bproj_bc = consts.tile([128, C], F32, name="bproj_bc")
                bfc2_bc = consts.tile([128, C], F32, name="bfc2_bc")
                nc.gpsimd.dma_start(out=g1_bc, in_=_bcast(ln1g_d[:]))
                nc.gpsimd.dma_start(out=b1_bc, in_=_bcast(ln1b_d[:]))
                nc.gpsimd.dma_start(out=g2_bc, in_=_bcast(ln2g_d[:]))
                nc.gpsimd.dma_start(out=b2_bc, in_=_bcast(ln2b_d[:]))
                nc.gpsimd.dma_start(out=battnv_bc, in_=_bcast(battn_d[ds(2 * C, C)]))
                nc.gpsimd.dma_start(out=bproj_bc, in_=_bcast(bproj_d[:]))
                nc.gpsimd.dma_start(out=bfc2_bc, in_=_bcast(bfc2_d[:]))
                # b_attn q/k biases arrive PERMUTED like the w_attn columns
                battn_qk = consts.tile([128, 12], F32, name="battn_qk")
                nc.gpsimd.dma_start(
                    out=battn_qk,
                    in_=battn_d[ds(0, 2 * C)].rearrange("(m p) -> p m", p=128),
                )
                bfc_col = consts.tile([128, FC], F32, name="bfc_col")
                nc.gpsimd.dma_start(
                    out=bfc_col, in_=bfc_d[:].rearrange("(m p) -> p m", p=128)
                )

            def layer_norm_to_hT(x_slice, g_bc, b_bc, hT, t, hT_lo=None):
                """LN over C (free dim) of one token tile; write transpose
                into hT[:, c, t*128:...] via PE transposes (copies on ScalarE).
                hT dtype (fp8 for h1T, bf16 for h2T) set by the copy cast."""
                stats = stat_pool.tile([128, 2, 6], F32, name="stats", tag="lnstats")
                for i in range(2):
                    nc.vector.bn_stats(out=stats[:, i, :], in_=x_slice[:, ts(i, 384)])
                mv = stat_pool.tile([128, 2], F32, name="mv", tag="lnmv")
                nc.vector.bn_aggr(out=mv, in_=stats)
                rstd = stat_pool.tile([128, 1], F32, name="rstd", tag="rstd")
                nc.scalar.activation(out=rstd, in_=mv[:, 1:2], func=AF.Sqrt, bias=eps_t[:, 0:1])
                nc.vector.reciprocal(rstd, rstd)
                hbf = htmp_pool.tile([128, C], BF16, name="hbf", tag="hbf")
                if trivial:
                    nc.vector.tensor_scalar(
                        out=hbf, in0=x_slice, scalar1=mv[:, 0:1], scalar2=rstd,
                        op0=ALU.subtract, op1=ALU.mult,
                    )
                else:
                    htmp = htmp_pool.tile([128, C], F32, name="htmp", tag="htmp")
                    nc.vector.tensor_scalar(
                        out=htmp, in0=x_slice, scalar1=mv[:, 0:1], scalar2=rstd,
                        op0=ALU.subtract, op1=ALU.mult,
                    )
                    nc.vector.tensor_mul(htmp, htmp, g_bc)
                    nc.vector.tensor_add(hbf, htmp, b_bc)
                for c in range(CC):
                    ptr = ps_tr.tile([128, 128], BF16, name="ptr", tag="tr")
                    nc.tensor.transpose(ptr, hbf[:, ts(c, 128)], ident)
                    nc.scalar.copy(hT[:, c, ts(t, 128)], ptr)
                    if hT_lo is not None:
                        # subnormal-fp8 residual of the fp8 cast (no rescale)
                        nc.vector.tensor_sub(
                            hT_lo[:, c, ts(t, 128)], ptr, hT[:, c, ts(t, 128)]
                        )

            # ================= one full block (repeatable) =================
            for _rep in range(repeat):
              # residual stream, token-major fp32 (per-tile DMAs so LN1 can
              # start as soon as the first rows land)
              x_t = xpool.tile([128, TT, C], F32, name="x_t", tag="x_t")
              for t in range(TT):
                  nc.sync.dma_start(out=x_t[:, t, :], in_=x_d[ts(t, 128), :])

              # ---- attention scope ----
              with ExitStack() as actx:
                wattn_pool = actx.enter_context(tc.tile_pool(name="wattn", bufs=1))
                if mlp_split:
                    ps_s = actx.enter_context(tc.tile_pool(name="ps_s", bufs=2, space="PSUM"))
                hT_pool = actx.enter_context(tc.tile_pool(name="hT1", bufs=1))
                qkT_pool = actx.enter_context(tc.tile_pool(name="qkT", bufs=1))
                vext_pool = actx.enter_context(tc.tile_pool(name="vext", bufs=1))
                sT_pool = actx.enter_context(tc.tile_pool(name="sT", bufs=3))
                opool = actx.enter_context(tc.tile_pool(name="opool", bufs=1))

                wattn_sb = wattn_pool.tile([128, CC, 3 * C], FP8, name="wattn_sb")
                nc.gpsimd.dma_start(
                    out=wattn_sb, in_=wattn_d[:, :].rearrange("(c p) n -> p c n", p=128)
                )
                wproj_sb = wproj_pool.tile([128, CC, C], FP8, name="wproj_sb")
                nc.gpsimd.dma_start(
                    out=wproj_sb, in_=wproj_d[:, :].rearrange("(c p) n -> p c n", p=128)
                )

                h1T = hT_pool.tile([128, CC, T], FP8, name="h1T", tag="hT")
                qT = qkT_pool.tile([128, CC, T], qk_dt, name="qT", tag="qT")
                kT = qkT_pool.tile([128, CC, KT * 128], qk_dt, name="kT", tag="kT")
                vext = vext_pool.tile([128, NH, KT, HD + 1], sv_dt, name="vext")
                nc.vector.memset(vext[:, :, :, HD : HD + 1], 1.0)
                if trivial:
                    # mask the ones column up front (v columns get the mask
                    # folded into their psum->sbuf copies below)
                    for t in range(KT):
                        nc.vector.tensor_scalar_mul(
                            out=vext[:, :, t, HD : HD + 1],
                            in0=vext[:, :, t, HD : HD + 1],
                            scalar1=mask_col[:, t : t + 1],
                        )

                # q^T / k^T feature-major, emitted token-half by token-half
                # so the first half's matmuls run while LN1 of tiles 4-7 is
                # still computing (the PE stream is in-order)
                def qk_phase(nqi):
                    for m in range(12):
                        dest = qT if m < 6 else kT
                        nlim = T if m < 6 else KT * 128
                        n0 = nqi * 512
                        nsz = min(512, nlim - n0)
                        if nsz <= 0:
                            continue
                        pq = ps_mm.tile([128, nsz], F32, name="pq", tag="mm")
                        if dr_qkv:
                            for c in range(0, CC, 2):
                                nc.tensor.matmul(
                                    pq,
                                    lhsT=wattn_sb[:, c : c + 2, ts(m, 128)],
                                    rhs=h1T[:, c : c + 2, ds(n0, nsz)],
                                    start=(c == 0),
                                    stop=(c == CC - 2),
                                    perf_mode=DR,
                                )
                        else:
                            for c in range(CC):
                                nc.tensor.matmul(
                                    pq,
                                    lhsT=wattn_sb[:, c, ts(m, 128)],
                                    rhs=h1T[:, c, ds(n0, nsz)],
                                    start=(c == 0),
                                    stop=(c == CC - 1),
                                )
                        if trivial:
                            nc.vector.tensor_copy(dest[:, m % 6, ds(n0, nsz)], pq)
                        else:
                            nc.scalar.activation(
                                out=dest[:, m % 6, ds(n0, nsz)], in_=pq,
                                func=AF.Identity, bias=battn_qk[:, m : m + 1],
                            )

                for t in range(4):
                    layer_norm_to_hT(
                        x_t[:, t, :],
                        None if trivial else g1_bc,
                        None if trivial else b1_bc,
                        h1T, t,
                    )
                qk_phase(0)
                for t in range(4, TT):
                    layer_norm_to_hT(
                        x_t[:, t, :],
                        None if trivial else g1_bc,
                        None if trivial else b1_bc,
                        h1T, t,
                    )
                qk_phase(1)

                sT_tiles = {}

                def emit_scores(h):
                    # scores^T for head h: DoubleRow over the head dim (64 =
                    # 32 partitions x 2 chunk-slots, from the host-side W_attn
                    # column permutation); one big [128, 2048] exp per pair of
                    # kt chunks (amortizes the ACT op overhead)
                    sT = sT_pool.tile([128, KT, T], sv_dt, name="sT", tag="sT")
                    sT_tiles[h] = sT
                    for kt in range(KT):
                        pss = ps_s.tile([128, 2, 512], F32, name="pss", tag="ss")
                        for nq in range(2):
                            if dr_scores:
                                g, i = divmod(h, 4)
                                r0 = 32 * i
                                nc.tensor.matmul(
                                    pss[:, nq, :],
                                    lhsT=kT[ds(r0, 32), ds(2 * g, 2), ts(kt, 128)],
                                    rhs=qT[ds(r0, 32), ds(2 * g, 2), ts(nq, 512)],
                                    start=True,
                                    stop=True,
                                    perf_mode=DR,
                                    # base_partition() rejects 96; rows<=32 may
                                    # sit at any quadrant via explicit position
                                    tile_position=(r0, 0),
                                )
                            else:
                                hc, hr = divmod(h, 2)
                                r0 = hr * 64
                                nc.tensor.matmul(
                                    pss[:, nq, :],
                                    lhsT=kT[ds(r0, 64), hc, ts(kt, 128)],
                                    rhs=qT[ds(r0, 64), hc, ts(nq, 512)],
                                    start=True,
                                    stop=True,
                                )
                        nc.scalar.activation(
                            out=sT[:, kt, :],
                            in_=pss.rearrange("p a b -> p (a b)"),
                            func=AF.Exp,
                            scale=0.125,
                            bias=0.0 if exp_bias is None else expb_t[:, 0:1],
                        )

                # prime the head pipeline (sT is triple-buffered)
                emit_scores(0)
                emit_scores(1)
                emit_scores(2)

                # v token-major, scattered per head into v_ext (batched copies)
                for t in range(KT):
                    for n0, nsz in ((0, 512), (512, 256)):
                        pv = ps_mm.tile([128, nsz], F32, name="pv", tag="mm")
                        if dr_qkv:
                            for c in range(0, CC, 2):
                                nc.tensor.matmul(
                                    pv,
                                    lhsT=h1T[:, c : c + 2, ts(t, 128)],
                                    rhs=wattn_sb[:, c : c + 2, ds(2 * C + n0, nsz)],
                                    start=(c == 0),
                                    stop=(c == CC - 2),
                                    perf_mode=DR,
                                )
                        else:
                            for c in range(CC):
                                nc.tensor.matmul(
                                    pv,
                                    lhsT=h1T[:, c, ts(t, 128)],
                                    rhs=wattn_sb[:, c, ds(2 * C + n0, nsz)],
                                    start=(c == 0),
                                    stop=(c == CC - 1),
                                )
                        h0, h1 = n0 // HD, (n0 + nsz) // HD
                        pv_h = pv.rearrange("p (h d) -> p h d", d=HD)
                        if trivial:
                            # mask folded into the psum->sbuf copy
                            nc.vector.tensor_scalar_mul(
                                out=vext[:, h0:h1, t, 0:HD], in0=pv_h,
                                scalar1=mask_col[:, t : t + 1],
                            )
                        else:
                            nc.vector.tensor_add(
                                out=vext[:, h0:h1, t, 0:HD], in0=pv_h,
                                in1=battnv_bc[:, ds(n0, nsz)].rearrange(
                                    "p (h d) -> p h d", d=HD
                                ),
                            )
                if not trivial:
                    # fold the key-padding mask into v_ext (incl. ones col)
                    for t in range(KT):
                        nc.vector.tensor_scalar_mul(
                            out=vext[:, :, t, :], in0=vext[:, :, t, :],
                            scalar1=mask_col[:, t : t + 1],
                        )

                o_t = opool.tile([128, TT, C], BF16, name="o_t", tag="op")
                oT = qkT_pool.tile([128, CC, T], FP8, name="oT", tag="oT")
                KT_PAIRS = KT // 2
                for h in range(NH):
                    sT = sT_tiles.pop(h)
                    for tq in range(TT):
                        pav = ps_mm.tile([128, HD + 1], F32, name="pav", tag="mm")
                        if dr_av:
                            for j in range(KT_PAIRS):
                                nc.tensor.matmul(
                                    pav,
                                    lhsT=sT[:, 2 * j : 2 * j + 2, ts(tq, 128)],
                                    rhs=vext[:, h, 2 * j : 2 * j + 2, :],
                                    start=(j == 0),
                                    stop=(KT % 2 == 0 and j == KT_PAIRS - 1),
                                    perf_mode=DR,
                                )
                            if KT % 2 == 1:
                                nc.tensor.matmul(
                                    pav,
                                    lhsT=sT[:, KT - 1, ts(tq, 128)],
                                    rhs=vext[:, h, KT - 1, :],
                                    start=False,
                                    stop=True,
                                )
                        else:
                            for kt in range(KT):
                                nc.tensor.matmul(
                                    pav,
                                    lhsT=sT[:, kt, ts(tq, 128)],
                                    rhs=vext[:, h, kt, :],
                                    start=(kt == 0),
                                    stop=(kt == KT - 1),
                                )
                        rec = stat_pool.tile([128, 1], F32, name="rec", tag="rec")
                        nc.vector.reciprocal(rec, pav[:, HD : HD + 1])
                        nc.vector.tensor_scalar_mul(
                            out=o_t[:, tq, ts(h, HD)], in0=pav[:, 0:HD], scalar1=rec
                        )
                    if h + 3 < NH:
                        emit_scores(h + 3)
                    if h % 2 == 1:
                        # heads 2c,2c+1 done -> transpose o chunk c now (PE has
                        # slack while ACT exps; copies on DVE, not the busy ACT)
                        c = h // 2
                        for t in range(TT):
                            ptr = ps_tr.tile([128, 128], BF16, name="ptr2", tag="tr")
                            nc.tensor.transpose(ptr, o_t[:, t, ts(c, 128)], ident)
                            nc.vector.tensor_copy(oT[:, c, ts(t, 128)], ptr)

                # proj + residual into x (fp32); LN2 + h2T per 4-tile group
                # so fcT's first half (needs h2T tiles 0-3) starts while the
                # second half of proj still runs
                h2_dt = FP8 if mlp_split else BF16
                h2T = hT2_pool.tile([128, CC, T], h2_dt, name="h2T", tag="hT2")
                h2T_lo = (
                    hT2_pool.tile([128, CC, T], FP8, name="h2Tlo", tag="hT2lo")
                    if mlp_split else None
                )
                for grp in range(2):
                    for t in range(grp * 4, grp * 4 + 4):
                        for n0, nsz in ((0, 512), (512, 256)):
                            pp = ps_mm.tile([128, nsz], F32, name="pp", tag="mm")
                            if dr_proj:
                                for c in range(0, CC, 2):
                                    nc.tensor.matmul(
                                        pp,
                                        lhsT=oT[:, c : c + 2, ts(t, 128)],
                                        rhs=wproj_sb[:, c : c + 2, ds(n0, nsz)],
                                        start=(c == 0),
                                        stop=(c == CC - 2),
                                        perf_mode=DR,
                                    )
                            else:
                                for c in range(CC):
                                    nc.tensor.matmul(
                                        pp,
                                        lhsT=oT[:, c, ts(t, 128)],
                                        rhs=wproj_sb[:, c, ds(n0, nsz)],
                                        start=(c == 0),
                                        stop=(c == CC - 1),
                                    )
                            if not trivial:
                                nc.vector.tensor_add(pp, pp, bproj_bc[:, ds(n0, nsz)])
                            nc.vector.tensor_add(
                                x_t[:, t, ds(n0, nsz)], x_t[:, t, ds(n0, nsz)], pp
                            )
                    for t in range(grp * 4, grp * 4 + 4):
                        layer_norm_to_hT(
                            x_t[:, t, :],
                            None if trivial else g2_bc,
                            None if trivial else b2_bc,
                            h2T, t, hT_lo=h2T_lo,
                        )

              # ---- MLP scope ----
              with ExitStack() as mctx:
                aT_pool = mctx.enter_context(tc.tile_pool(name="aT", bufs=1))
                outsb_pool = mctx.enter_context(tc.tile_pool(name="outsb", bufs=2))

                if not mlp_split:
                    wfc_pool = mctx.enter_context(tc.tile_pool(name="wfc", bufs=1))
                    wfc2_pool = mctx.enter_context(tc.tile_pool(name="wfc2", bufs=1))
                    wfc_sb = wfc_pool.tile([128, CC, 4 * C], BF16, name="wfc_sb")
                    nc.gpsimd.dma_start(
                        out=wfc_sb, in_=wfc_d[:, :].rearrange("(c p) n -> p c n", p=128)
                    )
                    wfc2_sb = wfc2_pool.tile([128, FC, C], BF16, name="wfc2_sb")
                    nc.gpsimd.dma_start(
                        out=wfc2_sb, in_=wfc2_d[:, :].rearrange("(m p) n -> p m n", p=128)
                    )

                    for half in range(2):
                        aT = aT_pool.tile([128, FC, 512], BF16, name="aT", tag="aT")
                        for m in range(FC):
                            pf = ps_mm.tile([128, 512], F32, name="pf", tag="mm")
                            for c in range(CC):
                                nc.tensor.matmul(
                                    pf,
                                    lhsT=wfc_sb[:, c, ts(m, 128)],
                                    rhs=h2T[:, c, ds(half * 512, 512)],
                                    start=(c == 0),
                                    stop=(c == CC - 1),
                                )
                            nc.scalar.activation(
                                out=aT[:, m, :], in_=pf, func=AF.Gelu_apprx_tanh,
                                bias=0.0 if trivial else bfc_col[:, m : m + 1],
                            )
                        for i in range(4):
                            t = half * 4 + i
                            outsb = outsb_pool.tile([128, C], F32, name="outsb", tag="outsb")
                            for n0, nsz in ((0, 512), (512, 256)):
                                pf2 = ps_mm.tile([128, nsz], F32, name="pf2", tag="mm")
                                for m in range(FC):
                                    nc.tensor.matmul(
                                        pf2,
                                        lhsT=aT[:, m, ts(i, 128)],
                                        rhs=wfc2_sb[:, m, ds(n0, nsz)],
                                        start=(m == 0),
                                        stop=(m == FC - 1),
                                    )
                                if not trivial:
                                    nc.vector.tensor_add(pf2, pf2, bfc2_bc[:, ds(n0, nsz)])
                                nc.vector.tensor_add(
                                    outsb[:, ds(n0, nsz)], x_t[:, t, ds(n0, nsz)], pf2
                                )
                            nc.sync.dma_start(out=out_d[ts(t, 128), :], in_=outsb)
                else:
                    # split-fp8 MLP: W stored as hi=fp8(16W), lo=fp8((16W-hi)*16).
                    # P1 accumulates scale-16 terms (hi@hi and act_lo@hi: act lo
                    # is the UNSCALED subnormal-fp8 cast residual), P2 the
                    # scale-256 term (hi@lo); combined = P1 + P2/16, and the
                    # final 1/16 folds into the gelu scale / output rescale.
                    wfc_pool = mctx.enter_context(tc.tile_pool(name="wfc", bufs=1))
                    wfc2_pool = mctx.enter_context(tc.tile_pool(name="wfc2", bufs=1))
                    a8_pool = mctx.enter_context(tc.tile_pool(name="a8", bufs=1))
                    ps_mlp = mctx.enter_context(
                        tc.tile_pool(name="ps_mlp", bufs=4, space="PSUM")
                    )
                    wfchi_sb = wfc_pool.tile([128, CC, 4 * C], FP8, name="wfchi_sb")
                    wfclo_sb = wfc_pool.tile([128, CC, 4 * C], FP8, name="wfclo_sb")
                    nc.gpsimd.dma_start(
                        out=wfchi_sb, in_=wfchi_d[:, :].rearrange("(c p) n -> p c n", p=128)
                    )
                    nc.gpsimd.dma_start(
                        out=wfclo_sb, in_=wfclo_d[:, :].rearrange("(c p) n -> p c n", p=128)
                    )
                    wfc2hi_sb = wfc2_pool.tile([128, FC, C], FP8, name="wfc2hi_sb")
                    wfc2lo_sb = wfc2_pool.tile([128, FC, C], FP8, name="wfc2lo_sb")
                    nc.gpsimd.dma_start(
                        out=wfc2hi_sb, in_=wfc2hi_d[:, :].rearrange("(m p) n -> p m n", p=128)
                    )
                    nc.gpsimd.dma_start(
                        out=wfc2lo_sb, in_=wfc2lo_d[:, :].rearrange("(m p) n -> p m n", p=128)
                    )

                    for half in range(2):
                        aT = aT_pool.tile([128, FC, 512], BF16, name="aT", tag="aT")
                        ahi = a8_pool.tile([128, FC, 512], FP8, name="ahi", tag="ahi")
                        alo = a8_pool.tile([128, FC, 512], FP8, name="alo", tag="alo")
                        hcols = ds(half * 512, 512)
                        for m in range(FC):
                            p1 = ps_mlp.tile([128, 512], F32, name="p1", tag="mmp")
                            for li, (wsb, hT_) in enumerate(
                                ((wfchi_sb, h2T), (wfclo_sb, h2T), (wfchi_sb, h2T_lo))
                            ):
                                for c in range(0, CC, 2):
                                    nc.tensor.matmul(
                                        p1,
                                        lhsT=wsb[:, c : c + 2, ts(m, 128)],
                                        rhs=hT_[:, c : c + 2, hcols],
                                        start=(li == 0 and c == 0),
                                        stop=(li == 2 and c == CC - 2),
                                        perf_mode=DR,
                                    )
                            nc.scalar.activation(
                                out=aT[:, m, :], in_=p1, func=AF.Gelu_apprx_tanh,
                                scale=1.0 / 16.0,
                                bias=0.0 if trivial else bfc_col[:, m : m + 1],
                            )
                            nc.vector.tensor_copy(ahi[:, m, :], aT[:, m, :])
                            nc.vector.tensor_sub(alo[:, m, :], aT[:, m, :], ahi[:, m, :])
                        for i in range(4):
                            t = half * 4 + i
                            outsb = outsb_pool.tile([128, C], F32, name="outsb", tag="outsb")
                            for n0, nsz in ((0, 512), (512, 256)):
                                q1 = ps_mlp.tile([128, nsz], F32, name="q1", tag="mmp")
                                for li, (aT_, wsb) in enumerate(
                                    ((ahi, wfc2hi_sb), (ahi, wfc2lo_sb), (alo, wfc2hi_sb))
                                ):
                                    for m in range(0, FC, 2):
                                        nc.tensor.matmul(
                                            q1,
                                            lhsT=aT_[:, m : m + 2, ts(i, 128)],
                                            rhs=wsb[:, m : m + 2, ds(n0, nsz)],
                                            start=(li == 0 and m == 0),
                                            stop=(li == 2 and m == FC - 2),
                                            perf_mode=DR,
                                        )
                                nc.vector.tensor_scalar_mul(
                                    outsb[:, ds(n0, nsz)], q1, sixt_t[:, 0:1]
                                )
                                nc.vector.tensor_add(
                                    outsb[:, ds(n0, nsz)],
                                    outsb[:, ds(n0, nsz)],
                                    x_t[:, t, ds(n0, nsz)],
                                )
                                if not trivial:
                                    nc.vector.tensor_add(
                                        outsb[:, ds(n0, nsz)],
                                        outsb[:, ds(n0, nsz)],
                                        bfc2_bc[:, ds(n0, nsz)],
                                    )
                            nc.sync.dma_start(out=out_d[ts(t, 128), :], in_=outsb)

    return nc


_NC_CACHE = {}

COMPACT_KT = 5  # attention processes 5*128 = 640 keys; guarded in kernel()

# Graded configuration: DoubleRow only where hardware-verified faster.
DR_QKV = True
DR_SCORES = False
DR_AV = False
DR_PROJ = True
MLP_SPLIT = False


def _get_nc(trivial: bool = True, kt_chunks: int = COMPACT_KT) -> bass.Bass:
    key = (trivial, kt_chunks)
    if key not in _NC_CACHE:
        nc = build_bass(
            trivial=trivial, kt_chunks=kt_chunks,
            dr_qkv=DR_QKV, dr_scores=DR_SCORES, dr_av=DR_AV, dr_proj=DR_PROJ,
            mlp_split=MLP_SPLIT,
        )
        nc.finalize()
        _NC_CACHE[key] = nc
    return _NC_CACHE[key]


TRACE = False
LAST_RESULTS = None
LAST_IN_MAPS = None


def _permute_qk_cols(w_qk: np.ndarray) -> np.ndarray:
    """Reorder q-or-k columns [.., 768] so head h's dims 0-31 / 32-63 land in
    feature chunks 2*(h//4) / 2*(h//4)+1 at partitions 32*(h%4)."""
    perm = np.empty(C, np.int64)
    pos = 0
    for g in range(3):           # head groups of 4
        for half in range(2):    # dims 0-31 | 32-63
            for i in range(4):   # head within group
                h = 4 * g + i
                perm[pos : pos + 32] = h * HD + half * 32 + np.arange(32)
                pos += 32
    return w_qk[..., perm]


def kernel(**inputs) -> np.ndarray:
    global LAST_RESULTS, LAST_IN_MAPS

    f32 = lambda a: np.ascontiguousarray(np.asarray(a, dtype=np.float32))
    bf = lambda a: np.ascontiguousarray(
        np.asarray(a, dtype=np.float32).astype(ml_dtypes.bfloat16)
    )
    f8 = lambda a: np.ascontiguousarray(
        np.asarray(a, dtype=np.float32).astype(ml_dtypes.float8_e4m3)
    )

    x = f32(inputs["x"])                       # [8, 1024, 768]
    mask = np.asarray(inputs["attn_mask"])     # [8, 1024] int32

    lng1, lnb1 = f32(inputs["ln1_g"]), f32(inputs["ln1_b"])
    lng2, lnb2 = f32(inputs["ln2_g"]), f32(inputs["ln2_b"])
    ba, bp = f32(inputs["b_attn"]), f32(inputs["b_proj"])
    bf_, bf2 = f32(inputs["b_fc"]), f32(inputs["b_fc2"])
    trivial = bool(
        (lng1 == 1).all() and (lnb1 == 0).all() and (lng2 == 1).all()
        and (lnb2 == 0).all() and (ba == 0).all() and (bp == 0).all()
        and (bf_ == 0).all() and (bf2 == 0).all()
    )

    # Key compaction: permute tokens per batch so unmasked keys come first.
    # Attention is permutation-equivariant over keys, and LN/MLP/residual are
    # per-token, so permuting rows of x and un-permuting the output is exact.
    # With <= COMPACT_KT*128 unmasked keys the remaining key chunks are all
    # masked (zero contribution) and can be skipped entirely.
    mask01 = (mask != 0)
    counts = mask01.sum(axis=1)
    compact = bool(counts.max() <= COMPACT_KT * 128)
    kt_chunks = COMPACT_KT if compact else TT

    perms = []
    for b in range(N_CORES):
        perm = np.argsort(~mask01[b], kind="stable")  # unmasked first
        perms.append(perm)

    nc = _get_nc(trivial, kt_chunks)

    # q/k column permutation for DoubleRow scores (see _permute_qk_cols)
    W_attn = f32(inputs["W_attn"])
    if DR_SCORES:
        W_attn_perm = np.concatenate(
            [
                _permute_qk_cols(W_attn[:, 0:C]),
                _permute_qk_cols(W_attn[:, C : 2 * C]),
                W_attn[:, 2 * C :],
            ],
            axis=1,
        )
    else:
        W_attn_perm = W_attn
    common = {
        "w_attn": f8(W_attn_perm),
        "w_proj": f8(inputs["W_proj"]),
    }
    if MLP_SPLIT:
        def _wsplit(w):
            ws = f32(w) * 16.0
            hi = np.asarray(ws, np.float32).astype(ml_dtypes.float8_e4m3)
            lo = ws - hi.astype(np.float32)   # raw residual (subnormal fp8)
            return np.ascontiguousarray(hi), np.ascontiguousarray(
                lo.astype(ml_dtypes.float8_e4m3)
            )
        common["w_fc_hi"], common["w_fc_lo"] = _wsplit(inputs["W_fc"])
        common["w_fc2_hi"], common["w_fc2_lo"] = _wsplit(inputs["W_fc2"])
    else:
        common["w_fc"] = bf(inputs["W_fc"])
        common["w_fc2"] = bf(inputs["W_fc2"])
    if not trivial:
        if DR_SCORES:
            ba_perm = np.concatenate(
                [_permute_qk_cols(ba[0:C]), _permute_qk_cols(ba[C : 2 * C]), ba[2 * C :]]
            )
        else:
            ba_perm = ba
        common.update(
            ln1_g=lng1, ln1_b=lnb1, ln2_g=lng2, ln2_b=lnb2,
            b_attn=ba_perm, b_proj=bp, b_fc=bf_, b_fc2=bf2,
        )
    in_maps = []
    for b in range(N_CORES):
        m = dict(common)
        m["x"] = np.ascontiguousarray(x[b][perms[b]])
        m["mask01"] = np.ascontiguousarray(mask01[b][perms[b]].astype(np.float32))
        in_maps.append(m)

    from concourse.bass_utils import run_bass_kernel_spmd

    LAST_IN_MAPS = in_maps
    res = run_bass_kernel_spmd(nc, in_maps, core_ids=list(range(N_CORES)), trace=TRACE)
    LAST_RESULTS = res
    out = np.empty((N_CORES, T, C), np.float32)
    for b in range(N_CORES):
        out[b, perms[b]] = np.asarray(res.results[b]["out"])
    return out
